# revision 14
# baseline (speedup 1.0000x reference)
"""Trainium2 Bass kernel for nn_CausalSimulationModule (gnn message passing).

next_state = entity + (sum_j A[i,j] * relu(s[j] + t[i] + b1)) @ W2 + b2
A = softmax((edge_logits + gumbel(u)) / tau)   (row-wise over sources j)

Key algebraic move: W2 is linear, so aggregate h over sources j FIRST
(h-agg is (B,N,D)), then apply W2 once -- this removes the (B,N,N,D)@W2
matmul entirely.  The irreducible work is the 134M-element
relu + A-weighted reduction, done as:
  per (i, b):  H[d, j] = relu(sb1T[d, j] + tT[d, i])   (one fused op,
               split between ScalarE (activation bias) and VectorE
               (tensor_scalar add+max, 4x bf16 mode) to balance engines)
               hagg[:, i] = sum_j H * A_repl_i          (one fused DVE
               tensor_tensor_reduce with accum_out)
A_repl_i (row i of A broadcast across the 128 d-partitions) is produced
for free by a stride-0 partition-broadcast DMA from an HBM bf16 scratch.

Sharding: pure target-node sharding. Core c owns i in [c*64, (c+1)*64)
and loops all 4 batches internally. No collectives needed.
"""

import sys

sys.path.insert(0, "/opt/trn_rl_repo")

from contextlib import ExitStack

import numpy as np

import concourse.bass as bass
import concourse.tile as tile
from concourse import mybir
from concourse.masks import make_identity
from concourse.bass_utils import run_bass_kernel_spmd

B, N, D = 4, 512, 128
M = 8            # NeuronCores
IS = N // M      # 64 target rows per core
TAU = 0.5
EPS = 1e-9
P = 128

FP32 = mybir.dt.float32
BF16 = mybir.dt.bfloat16
AF = mybir.ActivationFunctionType
ALU = mybir.AluOpType

# Fraction of the 256 (i, b) units routed through the ScalarE-assisted path
# (ACT relu -> DVE 2x bf16 multiply -> ACT copy+accum); the rest run as one
# fused custom DVE op. Balances ScalarE (~1427ns/unit) vs VectorE saving.
ACT_SHARE_NUM = 104
ACT_SHARE_DEN = 256


def _use_act(idx: int) -> bool:
    # Evenly interleaved split of units between the two paths.
    return (idx * ACT_SHARE_NUM) % ACT_SHARE_DEN < ACT_SHARE_NUM


def _register_relu_mul_reduce():
    """Author a custom fused DVE op:
        out = relu(in0 + s0) * in1 ; accum_out = s1 + sum_free(out)
    This collapses the whole per-(i,b) inner loop (bias-add, relu, A-weight
    multiply, reduction over sources) into ONE VectorE instruction.
    Registered by appending to concourse.dve_ops.OPS at runtime; the uop
    table is generated per-NEFF, so no firmware change is needed."""
    from operator import add as _add

    from concourse import dve_ops as dops
    from concourse.dve_spec import C0, C1, Spec, Src0, Src1, lower, relu
    from concourse.dve_uop import DveOpSpec

    name = "RELU_ADD_MUL_REDUCE_ANT"
    for o in dops.OPS:
        if o.name == name:
            return o

    def _ref(in0, in1, s0, s1, imm2):
        b = (np.maximum(in0.astype(np.float32) + s0, 0) * in1).astype(np.float32)
        return b, s1 + b.reshape(b.shape[0], -1).sum(axis=-1, keepdims=True)

    spec = Spec(body=relu(Src0 + C0) * Src1, accum=_add, accum_init=C1, reference=_ref)
    shas = {}
    for ver in ("v3", "v4"):
        shas[ver] = DveOpSpec(name=name, uops=lower(spec, ver=ver)).sha(ver)
    op = dops.DveOp(name, spec, subdim=False, uops_sha=shas)
    row = dops._CUSTOM_DVE_ROW_BASE + len(dops.OPS)
    dops.OPS.append(op)
    dops._SUB_OPCODE_FOR_NAME[name] = row
    dops.CUSTOM_DVE_SPECS[name] = spec
    return op


def _split_multi_waits(bir_bytes: bytes) -> bytes:
    """Walrus in this container allows at most ONE attached sync wait per
    instruction ("Too many sync wait commands").  Tile attaches several.
    Hoist extra waits onto standalone EventSemaphore instructions (which
    accept up to 2 waits each) inserted just before the instruction on the
    same engine stream.  Waits here are monotonic sem-ge waits, so
    satisfying them sequentially is equivalent to waiting on all at once.
    """
    import json

    bir = json.loads(bir_bytes)
    uid = 0
    for fn in bir["functions"]:
        for blk in fn["blocks"]:
            out = []
            for inst in blk["instructions"]:
                si = inst.get("sync_info")
                waits = (si or {}).get("on_wait") or []
                if len(waits) > 1:
                    extra, keep = waits[:-1], waits[-1:]
                    for k in range(0, len(extra), 1):
                        out.append(
                            {
                                "name": f"{inst['name']}-esw{uid}",
                                "opcode": "EventSemaphore",
                                "engine": inst["engine"],
                                "debug": inst.get("debug", 0),
                                "ins": [],
                                "outs": [],
                                "sync_info": {
                                    "on_update": [],
                                    "on_wait": extra[k : k + 1],
                                },
                            }
                        )
                        uid += 1
                    si["on_wait"] = keep
                out.append(inst)
            blk["instructions"] = out
    return json.dumps(bir).encode()


def build_nc() -> bass.Bass:
    rmr_op = _register_relu_mul_reduce()
    nc = bass.Bass()

    ent = nc.dram_tensor("entity", [B, N, D], FP32, kind="ExternalInput")
    ent_rows = nc.dram_tensor("entity_rows", [B, IS, D], FP32, kind="ExternalInput")
    edge_rows = nc.dram_tensor("edge_rows", [IS, N], FP32, kind="ExternalInput")
    u_rows = nc.dram_tensor("u_rows", [IS, N], FP32, kind="ExternalInput")
    Wp_d = nc.dram_tensor("Wp", [D, D], FP32, kind="ExternalInput")
    bp_d = nc.dram_tensor("bp", [D], FP32, kind="ExternalInput")
    W1s_d = nc.dram_tensor("W1s", [D, D], FP32, kind="ExternalInput")
    W1t_d = nc.dram_tensor("W1t", [D, D], FP32, kind="ExternalInput")
    b1_d = nc.dram_tensor("b1", [D], FP32, kind="ExternalInput")
    W2_d = nc.dram_tensor("W2", [D, D], FP32, kind="ExternalInput")
    b2_d = nc.dram_tensor("b2", [D], FP32, kind="ExternalInput")

    next_rows = nc.dram_tensor("next_rows", [B, IS, D], FP32, kind="ExternalOutput")
    a_rows = nc.dram_tensor("a_rows", [IS, N], FP32, kind="ExternalOutput")

    with tile.TileContext(nc) as tc, ExitStack() as ctx:
        consts = ctx.enter_context(tc.tile_pool(name="consts", bufs=1))
        perb = ctx.enter_context(tc.tile_pool(name="perb", bufs=1))
        work = ctx.enter_context(tc.tile_pool(name="work", bufs=2))
        astage = ctx.enter_context(tc.tile_pool(name="astage", bufs=2))
        arepl_pool = ctx.enter_context(tc.tile_pool(name="arepl", bufs=4))
        h_pool = ctx.enter_context(tc.tile_pool(name="h", bufs=4))
        psum = ctx.enter_context(tc.tile_pool(name="psum", bufs=3, space="PSUM"))
        psum_big = ctx.enter_context(
            tc.tile_pool(name="psum_big", bufs=2, space="PSUM")
        )
        dram = ctx.enter_context(tc.tile_pool(name="dram", bufs=1, space="DRAM"))

        # ---- constants ----
        ident = consts.tile([P, P], FP32, tag="ident")
        make_identity(nc, ident)

        Wp_s = consts.tile([D, D], FP32, tag="Wp")
        nc.gpsimd.dma_start(Wp_s[:], Wp_d[:, :])
        W1s_s = consts.tile([D, D], FP32, tag="W1s")
        nc.gpsimd.dma_start(W1s_s[:], W1s_d[:, :])
        W1t_s = consts.tile([D, D], FP32, tag="W1t")
        nc.gpsimd.dma_start(W1t_s[:], W1t_d[:, :])

        w2f = work.tile([D, D], FP32, tag="w2f")
        nc.gpsimd.dma_start(w2f[:], W2_d[:, :])
        W2_bf = consts.tile([D, D], BF16, tag="W2bf")
        nc.vector.tensor_copy(W2_bf[:], w2f[:])

        bp_col = consts.tile([D, 1], FP32, tag="bp")
        nc.gpsimd.dma_start(bp_col[:], bp_d[:])
        b1_col = consts.tile([D, 1], FP32, tag="b1")
        nc.gpsimd.dma_start(b1_col[:], b1_d[:])

        b2f = work.tile([1, D], FP32, tag="b2f")
        b2_row_ap = bass.AP(tensor=b2_d[:].tensor, offset=0, ap=[[0, 1], [1, D]])
        nc.gpsimd.dma_start(b2f[:], b2_row_ap)
        b2_bf = consts.tile([1, D], BF16, tag="b2bf")
        nc.vector.tensor_copy(b2_bf[:], b2f[:])
        ones_bf = consts.tile([1, IS], BF16, tag="ones")
        nc.vector.memset(ones_bf[:], 1.0)

        # ---- adjacency: A = softmax((edge + gumbel(u)) / tau), fp32 ----
        edge_t = astage.tile([IS, N], FP32, tag="edge")
        nc.gpsimd.dma_start(edge_t[:], edge_rows[:, :])
        u_t = astage.tile([IS, N], FP32, tag="u")
        nc.gpsimd.dma_start(u_t[:], u_rows[:, :])

        eps_col = consts.tile([IS, 1], FP32, tag="epscol")
        nc.vector.memset(eps_col[:], EPS)
        l1 = astage.tile([IS, N], FP32, tag="l1")
        nc.scalar.activation(l1[:], u_t[:], AF.Ln, bias=eps_col[:], scale=1.0)
        l2 = astage.tile([IS, N], FP32, tag="l2")
        # log(eps - l1) = log(-log(u+eps) + eps)
        nc.scalar.activation(l2[:], l1[:], AF.Ln, bias=eps_col[:], scale=-1.0)
        # z = (edge - l2) * (1/tau)
        zt = astage.tile([IS, N], FP32, tag="zt")
        nc.vector.tensor_sub(zt[:], edge_t[:], l2[:])
        z2 = astage.tile([IS, N], FP32, tag="z2")
        nc.vector.tensor_scalar_mul(z2[:], zt[:], 1.0 / TAU)

        mneg = astage.tile([IS, 1], FP32, tag="mneg")
        nc.vector.tensor_reduce(
            mneg[:], z2[:], axis=mybir.AxisListType.X, op=ALU.max, negate=True
        )
        ex = astage.tile([IS, N], FP32, tag="ex")
        zsum = astage.tile([IS, 1], FP32, tag="zsum")
        nc.scalar.activation(
            ex[:], z2[:], AF.Exp, bias=mneg[:], scale=1.0, accum_out=zsum[:]
        )
        zinv = astage.tile([IS, 1], FP32, tag="zinv")
        nc.vector.reciprocal(zinv[:], zsum[:])
        a_f32 = astage.tile([IS, N], FP32, tag="af32")
        nc.vector.tensor_scalar_mul(a_f32[:], ex[:], zinv[:])
        nc.gpsimd.dma_start(a_rows[:, :], a_f32[:])

        a_bf = astage.tile([IS, N], BF16, tag="abf")
        nc.vector.tensor_copy(a_bf[:], a_f32[:])
        a_scr = dram.tile([IS, N], BF16, tag="ascr")
        nc.gpsimd.dma_start(a_scr[:], a_bf[:])

        # ---- per-batch projection chain (PE, fp32) ----
        sb1T = []
        tT = []
        hagg = []
        er = []
        for b in range(B):
            # entity[b] transposed -> eT [d_in, 512]
            eT = work.tile([P, N], FP32, tag="eT")
            for k in range(4):
                ek = work.tile([P, D], FP32, tag="ek")
                nc.gpsimd.dma_start(ek[:], ent[b, k * P : (k + 1) * P, :])
                tp = psum.tile([P, P], FP32, tag="ps_small")
                nc.tensor.transpose(tp[:], ek[:], ident[:])
                nc.vector.tensor_copy(eT[:, k * P : (k + 1) * P], tp[:])

            er_b = perb.tile([IS, D], FP32, tag=f"er{b}")
            nc.gpsimd.dma_start(er_b[:], ent_rows[b, :, :])
            tpr = psum.tile([P, IS], FP32, tag="ps_small")
            nc.tensor.transpose(tpr[:], er_b[:], ident[:IS, :IS])
            eTr = work.tile([P, IS], FP32, tag="eTr")
            nc.vector.tensor_copy(eTr[:], tpr[:])

            pj = psum_big.tile([P, N], FP32, tag="ps_big")
            nc.tensor.matmul(pj[:], Wp_s[:], eT[:], start=True, stop=True)
            projT = work.tile([P, N], FP32, tag="projT")
            nc.scalar.activation(projT[:], pj[:], AF.Identity, bias=bp_col[:])

            pjr = psum.tile([P, IS], FP32, tag="ps_small")
            nc.tensor.matmul(pjr[:], Wp_s[:], eTr[:], start=True, stop=True)
            projTr = work.tile([P, IS], FP32, tag="projTr")
            nc.scalar.activation(projTr[:], pjr[:], AF.Identity, bias=bp_col[:])

            spx = psum_big.tile([P, N], FP32, tag="ps_big")
            nc.tensor.matmul(spx[:], W1s_s[:], projT[:], start=True, stop=True)
            sb1T_b = perb.tile([P, N], BF16, tag=f"sb1_{b}")
            nc.scalar.activation(sb1T_b[:], spx[:], AF.Identity, bias=b1_col[:])

            tpx = psum.tile([P, IS], FP32, tag="ps_small")
            nc.tensor.matmul(tpx[:], W1t_s[:], projTr[:], start=True, stop=True)
            tT_b = perb.tile([P, IS], FP32, tag=f"tT{b}")
            nc.vector.tensor_copy(tT_b[:], tpx[:])

            hagg_b = perb.tile([P, IS], FP32, tag=f"hagg{b}")

            sb1T.append(sb1T_b)
            tT.append(tT_b)
            hagg.append(hagg_b)
            er.append(er_b)

        # ---- main loop: relu + A-weighted aggregation over sources ----
        for i in range(IS):
            arp = arepl_pool.tile([P, N], BF16, tag="arp")
            nc.gpsimd.dma_start(arp[:], a_scr[i : i + 1, :].to_broadcast((P, N)))
            for b in range(B):
                idx = i * B + b
                if _use_act(idx):
                    # ScalarE-assisted path: ACT relu -> DVE 2x bf16 mult ->
                    # ACT copy with fused free-dim accumulation.
                    h = h_pool.tile([P, N], BF16, tag="h")
                    nc.scalar.activation(
                        h[:], sb1T[b][:], AF.Relu, bias=tT[b][:, i : i + 1]
                    )
                    p = h_pool.tile([P, N], BF16, tag="p")
                    nc.vector.tensor_mul(p[:], h[:], arp[:])
                    scr = h_pool.tile([P, N], BF16, tag="scr")
                    nc.scalar.activation(
                        scr[:],
                        p[:],
                        AF.Copy,
                        bias=0.0,
                        accum_out=hagg[b][:, i : i + 1],
                    )
                else:
                    # Fully fused: one VectorE instruction does bias-add,
                    # relu, A-weighting, and the reduction over sources.
                    hw = h_pool.tile([P, N], BF16, tag="hw")
                    nc.vector._custom_dve(
                        rmr_op,
                        out=hw[:],
                        in0=sb1T[b][:],
                        in1=arp[:],
                        s0=tT[b][:, i : i + 1],
                        s1=0.0,
                        accum_out=hagg[b][:, i : i + 1],
                    )

        # ---- tail: upd = hagg @ W2 + b2 ; next = entity_rows + upd ----
        for b in range(B):
            hagg_bf = work.tile([P, IS], BF16, tag="haggbf")
            nc.vector.tensor_copy(hagg_bf[:], hagg[b][:])
            updp = psum.tile([IS, D], FP32, tag="ps_small")
            nc.tensor.matmul(updp[:], hagg_bf[:], W2_bf[:], start=True, stop=False)
            nc.tensor.matmul(updp[:], ones_bf[:], b2_bf[:], start=False, stop=True)
            nextt = work.tile([IS, D], FP32, tag="nextt")
            nc.vector.tensor_add(nextt[:], er[b][:], updp[:])
            nc.gpsimd.dma_start(next_rows[b, :, :], nextt[:])

    # Lower InstISA subclasses (the custom DVE op) to raw instruction bytes;
    # Bacc.compile() does this but plain Bass+Tile does not.
    mybir.codegen_inst_isa_subclasses(nc)

    import types

    orig = type(nc).to_json_bytes
    nc.to_json_bytes = types.MethodType(
        lambda self: _split_multi_waits(orig(self)), nc
    )
    return nc


_CACHE: dict = {}


def kernel(entity_states, Wp, bp, edge_logits, W1, b1, W2, b2, u_noise):
    entity_states = np.ascontiguousarray(entity_states, dtype=np.float32)
    Wp = np.ascontiguousarray(Wp, dtype=np.float32)
    bp = np.ascontiguousarray(bp, dtype=np.float32)
    edge_logits = np.ascontiguousarray(edge_logits, dtype=np.float32)
    W1 = np.ascontiguousarray(W1, dtype=np.float32)
    b1 = np.ascontiguousarray(b1, dtype=np.float32)
    W2 = np.ascontiguousarray(W2, dtype=np.float32)
    b2 = np.ascontiguousarray(b2, dtype=np.float32)
    u_noise = np.ascontiguousarray(u_noise, dtype=np.float32)

    if "nc" not in _CACHE:
        _CACHE["nc"] = build_nc()
    nc = _CACHE["nc"]

    in_maps = []
    for c in range(M):
        sl = slice(c * IS, (c + 1) * IS)
        in_maps.append(
            {
                "entity": entity_states,
                "entity_rows": np.ascontiguousarray(entity_states[:, sl]),
                "edge_rows": np.ascontiguousarray(edge_logits[sl]),
                "u_rows": np.ascontiguousarray(u_noise[sl]),
                "Wp": Wp,
                "bp": bp,
                "W1s": np.ascontiguousarray(W1[:D]),
                "W1t": np.ascontiguousarray(W1[D:]),
                "b1": b1,
                "W2": W2,
                "b2": b2,
            }
        )

    res = run_bass_kernel_spmd(nc, in_maps, list(range(M)))
    results = res.results

    next_state = np.concatenate(
        [results[c]["next_rows"] for c in range(M)], axis=1
    ).astype(np.float32)
    a_full = np.concatenate([results[c]["a_rows"] for c in range(M)], axis=0).astype(
        np.float32
    )
    a_bcast = np.broadcast_to(a_full, (B, N, N)).copy()
    return next_state, a_bcast


# revision 15
# speedup vs baseline: 1.2042x; 1.2042x over previous
"""Trainium2 Bass kernel for nn_CausalSimulationModule (gnn message passing).

next_state = entity + (sum_j A[i,j] * relu(s[j] + t[i] + b1)) @ W2 + b2
A = softmax((edge_logits + gumbel(u)) / tau)   (row-wise over sources j)

Key algebraic move: W2 is linear, so aggregate h over sources j FIRST
(h-agg is (B,N,D)), then apply W2 once -- this removes the (B,N,N,D)@W2
matmul entirely.  The irreducible work is the 134M-element
relu + A-weighted reduction, done as:
  per (i, b):  H[d, j] = relu(sb1T[d, j] + tT[d, i])   (one fused op,
               split between ScalarE (activation bias) and VectorE
               (tensor_scalar add+max, 4x bf16 mode) to balance engines)
               hagg[:, i] = sum_j H * A_repl_i          (one fused DVE
               tensor_tensor_reduce with accum_out)
A_repl_i (row i of A broadcast across the 128 d-partitions) is produced
for free by a stride-0 partition-broadcast DMA from an HBM bf16 scratch.

Sharding: pure target-node sharding. Core c owns i in [c*64, (c+1)*64)
and loops all 4 batches internally. No collectives needed.
"""

import sys

sys.path.insert(0, "/opt/trn_rl_repo")

from contextlib import ExitStack

import numpy as np

import concourse.bass as bass
import concourse.tile as tile
from concourse import mybir
from concourse.masks import make_identity
from concourse.bass_utils import run_bass_kernel_spmd

B, N, D = 4, 512, 128
M = 8            # NeuronCores
IS = N // M      # 64 target rows per core
TAU = 0.5
EPS = 1e-9
P = 128

FP32 = mybir.dt.float32
BF16 = mybir.dt.bfloat16
AF = mybir.ActivationFunctionType
ALU = mybir.AluOpType

# Fraction of the 256 (i, b) units routed through the ScalarE-assisted path
# (ACT relu -> DVE 2x bf16 multiply -> ACT copy+accum); the rest run as one
# fused custom DVE op. Balances ScalarE (~1427ns/unit) vs VectorE saving.
ACT_SHARE_NUM = 104
ACT_SHARE_DEN = 256


def _use_act(idx: int) -> bool:
    # Evenly interleaved split of units between the two paths.
    return (idx * ACT_SHARE_NUM) % ACT_SHARE_DEN < ACT_SHARE_NUM


def _register_relu_mul_reduce():
    """Author a custom fused DVE op:
        out = relu(in0 + s0) * in1 ; accum_out = s1 + sum_free(out)
    This collapses the whole per-(i,b) inner loop (bias-add, relu, A-weight
    multiply, reduction over sources) into ONE VectorE instruction.
    Registered by appending to concourse.dve_ops.OPS at runtime; the uop
    table is generated per-NEFF, so no firmware change is needed."""
    from operator import add as _add

    from concourse import dve_ops as dops
    from concourse.dve_spec import C0, C1, Spec, Src0, Src1, lower, relu
    from concourse.dve_uop import DveOpSpec

    name = "RELU_ADD_MUL_REDUCE_ANT"
    for o in dops.OPS:
        if o.name == name:
            return o

    def _ref(in0, in1, s0, s1, imm2):
        b = (np.maximum(in0.astype(np.float32) + s0, 0) * in1).astype(np.float32)
        return b, s1 + b.reshape(b.shape[0], -1).sum(axis=-1, keepdims=True)

    spec = Spec(body=relu(Src0 + C0) * Src1, accum=_add, accum_init=C1, reference=_ref)
    shas = {}
    for ver in ("v3", "v4"):
        shas[ver] = DveOpSpec(name=name, uops=lower(spec, ver=ver)).sha(ver)
    op = dops.DveOp(name, spec, subdim=False, uops_sha=shas)
    row = dops._CUSTOM_DVE_ROW_BASE + len(dops.OPS)
    dops.OPS.append(op)
    dops._SUB_OPCODE_FOR_NAME[name] = row
    dops.CUSTOM_DVE_SPECS[name] = spec
    return op


def _split_multi_waits(bir_bytes: bytes) -> bytes:
    """Walrus in this container allows at most ONE attached sync wait per
    instruction ("Too many sync wait commands").  Tile attaches several.
    Hoist extra waits onto standalone EventSemaphore instructions (which
    accept up to 2 waits each) inserted just before the instruction on the
    same engine stream.  Waits here are monotonic sem-ge waits, so
    satisfying them sequentially is equivalent to waiting on all at once.
    """
    import json

    bir = json.loads(bir_bytes)
    uid = 0
    for fn in bir["functions"]:
        for blk in fn["blocks"]:
            out = []
            for inst in blk["instructions"]:
                si = inst.get("sync_info")
                waits = (si or {}).get("on_wait") or []
                if len(waits) > 1:
                    extra, keep = waits[:-1], waits[-1:]
                    for k in range(0, len(extra), 1):
                        out.append(
                            {
                                "name": f"{inst['name']}-esw{uid}",
                                "opcode": "EventSemaphore",
                                "engine": inst["engine"],
                                "debug": inst.get("debug", 0),
                                "ins": [],
                                "outs": [],
                                "sync_info": {
                                    "on_update": [],
                                    "on_wait": extra[k : k + 1],
                                },
                            }
                        )
                        uid += 1
                    si["on_wait"] = keep
                out.append(inst)
            blk["instructions"] = out
    return json.dumps(bir).encode()


def build_nc() -> bass.Bass:
    rmr_op = _register_relu_mul_reduce()
    nc = bass.Bass()

    ent = nc.dram_tensor("entity", [B, N, D], FP32, kind="ExternalInput")
    ent_rows = nc.dram_tensor("entity_rows", [B, IS, D], FP32, kind="ExternalInput")
    edge_rows = nc.dram_tensor("edge_rows", [IS, N], FP32, kind="ExternalInput")
    u_rows = nc.dram_tensor("u_rows", [IS, N], FP32, kind="ExternalInput")
    Wp_d = nc.dram_tensor("Wp", [D, D], FP32, kind="ExternalInput")
    bp_d = nc.dram_tensor("bp", [D], FP32, kind="ExternalInput")
    W1s_d = nc.dram_tensor("W1s", [D, D], FP32, kind="ExternalInput")
    W1t_d = nc.dram_tensor("W1t", [D, D], FP32, kind="ExternalInput")
    b1_d = nc.dram_tensor("b1", [D], FP32, kind="ExternalInput")
    W2_d = nc.dram_tensor("W2", [D, D], FP32, kind="ExternalInput")
    b2_d = nc.dram_tensor("b2", [D], FP32, kind="ExternalInput")

    next_rows = nc.dram_tensor("next_rows", [B, IS, D], FP32, kind="ExternalOutput")
    a_rows = nc.dram_tensor("a_rows", [IS, N], FP32, kind="ExternalOutput")

    with tile.TileContext(nc) as tc, ExitStack() as ctx:
        consts = ctx.enter_context(tc.tile_pool(name="consts", bufs=1))
        perb = ctx.enter_context(tc.tile_pool(name="perb", bufs=1))
        work = ctx.enter_context(tc.tile_pool(name="work", bufs=2))
        astage = ctx.enter_context(tc.tile_pool(name="astage", bufs=2))
        arepl_pool = ctx.enter_context(tc.tile_pool(name="arepl", bufs=6))
        h_pool = ctx.enter_context(tc.tile_pool(name="h", bufs=4))
        psum = ctx.enter_context(tc.tile_pool(name="psum", bufs=3, space="PSUM"))
        psum_big = ctx.enter_context(
            tc.tile_pool(name="psum_big", bufs=2, space="PSUM")
        )
        dram = ctx.enter_context(tc.tile_pool(name="dram", bufs=1, space="DRAM"))

        # ---- constants ----
        ident = consts.tile([P, P], FP32, tag="ident")
        make_identity(nc, ident)

        Wp_s = consts.tile([D, D], FP32, tag="Wp")
        nc.gpsimd.dma_start(Wp_s[:], Wp_d[:, :])
        W1s_s = consts.tile([D, D], FP32, tag="W1s")
        nc.gpsimd.dma_start(W1s_s[:], W1s_d[:, :])
        W1t_s = consts.tile([D, D], FP32, tag="W1t")
        nc.gpsimd.dma_start(W1t_s[:], W1t_d[:, :])

        w2f = work.tile([D, D], FP32, tag="w2f")
        nc.gpsimd.dma_start(w2f[:], W2_d[:, :])
        W2_bf = consts.tile([D, D], BF16, tag="W2bf")
        nc.vector.tensor_copy(W2_bf[:], w2f[:])

        bp_col = consts.tile([D, 1], FP32, tag="bp")
        nc.gpsimd.dma_start(bp_col[:], bp_d[:])
        b1_col = consts.tile([D, 1], FP32, tag="b1")
        nc.gpsimd.dma_start(b1_col[:], b1_d[:])

        b2f = work.tile([1, D], FP32, tag="b2f")
        b2_row_ap = bass.AP(tensor=b2_d[:].tensor, offset=0, ap=[[0, 1], [1, D]])
        nc.gpsimd.dma_start(b2f[:], b2_row_ap)
        b2_bf = consts.tile([1, D], BF16, tag="b2bf")
        nc.vector.tensor_copy(b2_bf[:], b2f[:])
        ones_bf = consts.tile([1, IS], BF16, tag="ones")
        nc.vector.memset(ones_bf[:], 1.0)

        # ---- adjacency: A = softmax((edge + gumbel(u)) / tau), fp32 ----
        edge_t = astage.tile([IS, N], FP32, tag="edge")
        nc.gpsimd.dma_start(edge_t[:], edge_rows[:, :])
        u_t = astage.tile([IS, N], FP32, tag="u")
        nc.gpsimd.dma_start(u_t[:], u_rows[:, :])

        eps_col = consts.tile([IS, 1], FP32, tag="epscol")
        nc.vector.memset(eps_col[:], EPS)
        l1 = astage.tile([IS, N], FP32, tag="l1")
        nc.scalar.activation(l1[:], u_t[:], AF.Ln, bias=eps_col[:], scale=1.0)
        l2 = astage.tile([IS, N], FP32, tag="l2")
        # log(eps - l1) = log(-log(u+eps) + eps)
        nc.scalar.activation(l2[:], l1[:], AF.Ln, bias=eps_col[:], scale=-1.0)
        # z = (edge - l2) * (1/tau)
        zt = astage.tile([IS, N], FP32, tag="zt")
        nc.vector.tensor_sub(zt[:], edge_t[:], l2[:])
        z2 = astage.tile([IS, N], FP32, tag="z2")
        nc.vector.tensor_scalar_mul(z2[:], zt[:], 1.0 / TAU)

        mneg = astage.tile([IS, 1], FP32, tag="mneg")
        nc.vector.tensor_reduce(
            mneg[:], z2[:], axis=mybir.AxisListType.X, op=ALU.max, negate=True
        )
        ex = astage.tile([IS, N], FP32, tag="ex")
        zsum = astage.tile([IS, 1], FP32, tag="zsum")
        nc.scalar.activation(
            ex[:], z2[:], AF.Exp, bias=mneg[:], scale=1.0, accum_out=zsum[:]
        )
        zinv = astage.tile([IS, 1], FP32, tag="zinv")
        nc.vector.reciprocal(zinv[:], zsum[:])
        a_f32 = astage.tile([IS, N], FP32, tag="af32")
        nc.vector.tensor_scalar_mul(a_f32[:], ex[:], zinv[:])
        nc.sync.dma_start(a_rows[:, :], a_f32[:])

        a_bf = astage.tile([IS, N], BF16, tag="abf")
        nc.vector.tensor_copy(a_bf[:], a_f32[:])
        a_scr = dram.tile([IS, N], BF16, tag="ascr")
        nc.gpsimd.dma_start(a_scr[:], a_bf[:])

        # ---- per-batch projection chain (PE, fp32) ----
        sb1T = []
        tT = []
        hagg = []
        er = []
        for b in range(B):
            # entity[b] transposed -> eT [d_in, 512]
            eT = work.tile([P, N], FP32, tag="eT")
            for k in range(4):
                ek = work.tile([P, D], FP32, tag="ek")
                nc.sync.dma_start(ek[:], ent[b, k * P : (k + 1) * P, :])
                tp = psum.tile([P, P], FP32, tag="ps_small")
                nc.tensor.transpose(tp[:], ek[:], ident[:])
                nc.vector.tensor_copy(eT[:, k * P : (k + 1) * P], tp[:])

            er_b = perb.tile([IS, D], FP32, tag=f"er{b}")
            nc.sync.dma_start(er_b[:], ent_rows[b, :, :])
            tpr = psum.tile([P, IS], FP32, tag="ps_small")
            nc.tensor.transpose(tpr[:], er_b[:], ident[:IS, :IS])
            eTr = work.tile([P, IS], FP32, tag="eTr")
            nc.vector.tensor_copy(eTr[:], tpr[:])

            pj = psum_big.tile([P, N], FP32, tag="ps_big")
            nc.tensor.matmul(pj[:], Wp_s[:], eT[:], start=True, stop=True)
            projT = work.tile([P, N], FP32, tag="projT")
            nc.scalar.activation(projT[:], pj[:], AF.Identity, bias=bp_col[:])

            pjr = psum.tile([P, IS], FP32, tag="ps_small")
            nc.tensor.matmul(pjr[:], Wp_s[:], eTr[:], start=True, stop=True)
            projTr = work.tile([P, IS], FP32, tag="projTr")
            nc.scalar.activation(projTr[:], pjr[:], AF.Identity, bias=bp_col[:])

            spx = psum_big.tile([P, N], FP32, tag="ps_big")
            nc.tensor.matmul(spx[:], W1s_s[:], projT[:], start=True, stop=True)
            sb1T_b = perb.tile([P, N], BF16, tag=f"sb1_{b}")
            nc.scalar.activation(sb1T_b[:], spx[:], AF.Identity, bias=b1_col[:])

            tpx = psum.tile([P, IS], FP32, tag="ps_small")
            nc.tensor.matmul(tpx[:], W1t_s[:], projTr[:], start=True, stop=True)
            tT_b = perb.tile([P, IS], FP32, tag=f"tT{b}")
            nc.vector.tensor_copy(tT_b[:], tpx[:])

            hagg_b = perb.tile([P, IS], FP32, tag=f"hagg{b}")

            sb1T.append(sb1T_b)
            tT.append(tT_b)
            hagg.append(hagg_b)
            er.append(er_b)

        # ---- main loop: relu + A-weighted aggregation over sources ----
        for i in range(IS):
            arp = arepl_pool.tile([P, N], BF16, tag="arp")
            nc.sync.dma_start(arp[:], a_scr[i : i + 1, :].to_broadcast((P, N)))
            for b in range(B):
                idx = i * B + b
                if _use_act(idx):
                    # ScalarE-assisted path: ACT relu -> DVE 2x bf16 mult ->
                    # ACT copy with fused free-dim accumulation.
                    h = h_pool.tile([P, N], BF16, tag="h")
                    nc.scalar.activation(
                        h[:], sb1T[b][:], AF.Relu, bias=tT[b][:, i : i + 1]
                    )
                    p = h_pool.tile([P, N], BF16, tag="p")
                    nc.vector.tensor_mul(p[:], h[:], arp[:])
                    scr = h_pool.tile([P, N], BF16, tag="scr")
                    nc.scalar.activation(
                        scr[:],
                        p[:],
                        AF.Copy,
                        bias=0.0,
                        accum_out=hagg[b][:, i : i + 1],
                    )
                else:
                    # Fully fused: one VectorE instruction does bias-add,
                    # relu, A-weighting, and the reduction over sources.
                    hw = h_pool.tile([P, N], BF16, tag="hw")
                    nc.vector._custom_dve(
                        rmr_op,
                        out=hw[:],
                        in0=sb1T[b][:],
                        in1=arp[:],
                        s0=tT[b][:, i : i + 1],
                        s1=0.0,
                        accum_out=hagg[b][:, i : i + 1],
                    )

        # ---- tail: upd = hagg @ W2 + b2 ; next = entity_rows + upd ----
        for b in range(B):
            hagg_bf = work.tile([P, IS], BF16, tag="haggbf")
            nc.vector.tensor_copy(hagg_bf[:], hagg[b][:])
            updp = psum.tile([IS, D], FP32, tag="ps_small")
            nc.tensor.matmul(updp[:], hagg_bf[:], W2_bf[:], start=True, stop=False)
            nc.tensor.matmul(updp[:], ones_bf[:], b2_bf[:], start=False, stop=True)
            nextt = work.tile([IS, D], FP32, tag="nextt")
            nc.vector.tensor_add(nextt[:], er[b][:], updp[:])
            nc.sync.dma_start(next_rows[b, :, :], nextt[:])

    # Lower InstISA subclasses (the custom DVE op) to raw instruction bytes;
    # Bacc.compile() does this but plain Bass+Tile does not.
    mybir.codegen_inst_isa_subclasses(nc)

    import types

    orig = type(nc).to_json_bytes
    nc.to_json_bytes = types.MethodType(
        lambda self: _split_multi_waits(orig(self)), nc
    )
    return nc


_CACHE: dict = {}


def kernel(entity_states, Wp, bp, edge_logits, W1, b1, W2, b2, u_noise):
    entity_states = np.ascontiguousarray(entity_states, dtype=np.float32)
    Wp = np.ascontiguousarray(Wp, dtype=np.float32)
    bp = np.ascontiguousarray(bp, dtype=np.float32)
    edge_logits = np.ascontiguousarray(edge_logits, dtype=np.float32)
    W1 = np.ascontiguousarray(W1, dtype=np.float32)
    b1 = np.ascontiguousarray(b1, dtype=np.float32)
    W2 = np.ascontiguousarray(W2, dtype=np.float32)
    b2 = np.ascontiguousarray(b2, dtype=np.float32)
    u_noise = np.ascontiguousarray(u_noise, dtype=np.float32)

    if "nc" not in _CACHE:
        _CACHE["nc"] = build_nc()
    nc = _CACHE["nc"]

    in_maps = []
    for c in range(M):
        sl = slice(c * IS, (c + 1) * IS)
        in_maps.append(
            {
                "entity": entity_states,
                "entity_rows": np.ascontiguousarray(entity_states[:, sl]),
                "edge_rows": np.ascontiguousarray(edge_logits[sl]),
                "u_rows": np.ascontiguousarray(u_noise[sl]),
                "Wp": Wp,
                "bp": bp,
                "W1s": np.ascontiguousarray(W1[:D]),
                "W1t": np.ascontiguousarray(W1[D:]),
                "b1": b1,
                "W2": W2,
                "b2": b2,
            }
        )

    res = run_bass_kernel_spmd(nc, in_maps, list(range(M)))
    results = res.results

    next_state = np.concatenate(
        [results[c]["next_rows"] for c in range(M)], axis=1
    ).astype(np.float32)
    a_full = np.concatenate([results[c]["a_rows"] for c in range(M)], axis=0).astype(
        np.float32
    )
    a_bcast = np.broadcast_to(a_full, (B, N, N)).copy()
    return next_state, a_bcast


# revision 16
# speedup vs baseline: 1.2111x; 1.0057x over previous
"""Trainium2 Bass kernel for nn_CausalSimulationModule (gnn message passing).

next_state = entity + (sum_j A[i,j] * relu(s[j] + t[i] + b1)) @ W2 + b2
A = softmax((edge_logits + gumbel(u)) / tau)   (row-wise over sources j)

Key algebraic move: W2 is linear, so aggregate h over sources j FIRST
(h-agg is (B,N,D)), then apply W2 once -- this removes the (B,N,N,D)@W2
matmul entirely.  The irreducible work is the 134M-element
relu + A-weighted reduction, done as:
  per (i, b):  H[d, j] = relu(sb1T[d, j] + tT[d, i])   (one fused op,
               split between ScalarE (activation bias) and VectorE
               (tensor_scalar add+max, 4x bf16 mode) to balance engines)
               hagg[:, i] = sum_j H * A_repl_i          (one fused DVE
               tensor_tensor_reduce with accum_out)
A_repl_i (row i of A broadcast across the 128 d-partitions) is produced
for free by a stride-0 partition-broadcast DMA from an HBM bf16 scratch.

Sharding: pure target-node sharding. Core c owns i in [c*64, (c+1)*64)
and loops all 4 batches internally. No collectives needed.
"""

import sys

sys.path.insert(0, "/opt/trn_rl_repo")

from contextlib import ExitStack

import numpy as np

import concourse.bass as bass
import concourse.tile as tile
from concourse import mybir
from concourse.masks import make_identity
from concourse.bass_utils import run_bass_kernel_spmd

B, N, D = 4, 512, 128
M = 8            # NeuronCores
IS = N // M      # 64 target rows per core
TAU = 0.5
EPS = 1e-9
P = 128

FP32 = mybir.dt.float32
BF16 = mybir.dt.bfloat16
AF = mybir.ActivationFunctionType
ALU = mybir.AluOpType

# Fraction of the 256 (i, b) units routed through the ScalarE-assisted path
# (ACT relu -> DVE 2x bf16 multiply -> ACT copy+accum); the rest run as one
# fused custom DVE op. Balances ScalarE (~1427ns/unit) vs VectorE saving.
ACT_SHARE_NUM = 104
ACT_SHARE_DEN = 256


def _use_act(idx: int) -> bool:
    # Evenly interleaved split of units between the two paths.
    return (idx * ACT_SHARE_NUM) % ACT_SHARE_DEN < ACT_SHARE_NUM


def _register_relu_mul_reduce():
    """Author a custom fused DVE op:
        out = relu(in0 + s0) * in1 ; accum_out = s1 + sum_free(out)
    This collapses the whole per-(i,b) inner loop (bias-add, relu, A-weight
    multiply, reduction over sources) into ONE VectorE instruction.
    Registered by appending to concourse.dve_ops.OPS at runtime; the uop
    table is generated per-NEFF, so no firmware change is needed."""
    from operator import add as _add

    from concourse import dve_ops as dops
    from concourse.dve_spec import C0, C1, Spec, Src0, Src1, lower, relu
    from concourse.dve_uop import DveOpSpec

    name = "RELU_ADD_MUL_REDUCE_ANT"
    for o in dops.OPS:
        if o.name == name:
            return o

    def _ref(in0, in1, s0, s1, imm2):
        b = (np.maximum(in0.astype(np.float32) + s0, 0) * in1).astype(np.float32)
        return b, s1 + b.reshape(b.shape[0], -1).sum(axis=-1, keepdims=True)

    spec = Spec(body=relu(Src0 + C0) * Src1, accum=_add, accum_init=C1, reference=_ref)
    shas = {}
    for ver in ("v3", "v4"):
        shas[ver] = DveOpSpec(name=name, uops=lower(spec, ver=ver)).sha(ver)
    op = dops.DveOp(name, spec, subdim=False, uops_sha=shas)
    row = dops._CUSTOM_DVE_ROW_BASE + len(dops.OPS)
    dops.OPS.append(op)
    dops._SUB_OPCODE_FOR_NAME[name] = row
    dops.CUSTOM_DVE_SPECS[name] = spec
    return op


def _split_multi_waits(bir_bytes: bytes) -> bytes:
    """Walrus in this container allows at most ONE attached sync wait per
    instruction ("Too many sync wait commands").  Tile attaches several.
    Hoist extra waits onto standalone EventSemaphore instructions (which
    accept up to 2 waits each) inserted just before the instruction on the
    same engine stream.  Waits here are monotonic sem-ge waits, so
    satisfying them sequentially is equivalent to waiting on all at once.
    """
    import json

    bir = json.loads(bir_bytes)
    uid = 0
    for fn in bir["functions"]:
        for blk in fn["blocks"]:
            out = []
            for inst in blk["instructions"]:
                si = inst.get("sync_info")
                waits = (si or {}).get("on_wait") or []
                if len(waits) > 1:
                    extra, keep = waits[:-1], waits[-1:]
                    for k in range(0, len(extra), 1):
                        out.append(
                            {
                                "name": f"{inst['name']}-esw{uid}",
                                "opcode": "EventSemaphore",
                                "engine": inst["engine"],
                                "debug": inst.get("debug", 0),
                                "ins": [],
                                "outs": [],
                                "sync_info": {
                                    "on_update": [],
                                    "on_wait": extra[k : k + 1],
                                },
                            }
                        )
                        uid += 1
                    si["on_wait"] = keep
                out.append(inst)
            blk["instructions"] = out
    return json.dumps(bir).encode()


def build_nc() -> bass.Bass:
    rmr_op = _register_relu_mul_reduce()
    nc = bass.Bass()

    ent = nc.dram_tensor("entity", [B, N, D], FP32, kind="ExternalInput")
    ent_rows = nc.dram_tensor("entity_rows", [B, IS, D], FP32, kind="ExternalInput")
    edge_rows = nc.dram_tensor("edge_rows", [IS, N], FP32, kind="ExternalInput")
    u_rows = nc.dram_tensor("u_rows", [IS, N], FP32, kind="ExternalInput")
    Wp_d = nc.dram_tensor("Wp", [D, D], FP32, kind="ExternalInput")
    bp_d = nc.dram_tensor("bp", [D], FP32, kind="ExternalInput")
    W1s_d = nc.dram_tensor("W1s", [D, D], FP32, kind="ExternalInput")
    W1t_d = nc.dram_tensor("W1t", [D, D], FP32, kind="ExternalInput")
    b1_d = nc.dram_tensor("b1", [D], FP32, kind="ExternalInput")
    W2_d = nc.dram_tensor("W2", [D, D], FP32, kind="ExternalInput")
    b2_d = nc.dram_tensor("b2", [D], FP32, kind="ExternalInput")

    next_rows = nc.dram_tensor("next_rows", [B, IS, D], FP32, kind="ExternalOutput")
    a_rows = nc.dram_tensor("a_rows", [IS, N], FP32, kind="ExternalOutput")

    with tile.TileContext(nc) as tc, ExitStack() as ctx:
        consts = ctx.enter_context(tc.tile_pool(name="consts", bufs=1))
        perb = ctx.enter_context(tc.tile_pool(name="perb", bufs=1))
        work = ctx.enter_context(tc.tile_pool(name="work", bufs=2))
        astage = ctx.enter_context(tc.tile_pool(name="astage", bufs=2))
        arepl_pool = ctx.enter_context(tc.tile_pool(name="arepl", bufs=8))
        h_pool = ctx.enter_context(tc.tile_pool(name="h", bufs=8))
        psum = ctx.enter_context(tc.tile_pool(name="psum", bufs=3, space="PSUM"))
        psum_big = ctx.enter_context(
            tc.tile_pool(name="psum_big", bufs=2, space="PSUM")
        )
        dram = ctx.enter_context(tc.tile_pool(name="dram", bufs=1, space="DRAM"))

        # ---- constants ----
        ident = consts.tile([P, P], FP32, tag="ident")
        make_identity(nc, ident)

        Wp_s = consts.tile([D, D], FP32, tag="Wp")
        nc.gpsimd.dma_start(Wp_s[:], Wp_d[:, :])
        W1s_s = consts.tile([D, D], FP32, tag="W1s")
        nc.gpsimd.dma_start(W1s_s[:], W1s_d[:, :])
        W1t_s = consts.tile([D, D], FP32, tag="W1t")
        nc.gpsimd.dma_start(W1t_s[:], W1t_d[:, :])

        w2f = work.tile([D, D], FP32, tag="w2f")
        nc.gpsimd.dma_start(w2f[:], W2_d[:, :])
        W2_bf = consts.tile([D, D], BF16, tag="W2bf")
        nc.vector.tensor_copy(W2_bf[:], w2f[:])

        bp_col = consts.tile([D, 1], FP32, tag="bp")
        nc.gpsimd.dma_start(bp_col[:], bp_d[:])
        b1_col = consts.tile([D, 1], FP32, tag="b1")
        nc.gpsimd.dma_start(b1_col[:], b1_d[:])

        b2f = work.tile([1, D], FP32, tag="b2f")
        b2_row_ap = bass.AP(tensor=b2_d[:].tensor, offset=0, ap=[[0, 1], [1, D]])
        nc.gpsimd.dma_start(b2f[:], b2_row_ap)
        b2_bf = consts.tile([1, D], BF16, tag="b2bf")
        nc.vector.tensor_copy(b2_bf[:], b2f[:])
        ones_bf = consts.tile([1, IS], BF16, tag="ones")
        nc.vector.memset(ones_bf[:], 1.0)

        # ---- adjacency: A = softmax((edge + gumbel(u)) / tau), fp32 ----
        edge_t = astage.tile([IS, N], FP32, tag="edge")
        nc.gpsimd.dma_start(edge_t[:], edge_rows[:, :])
        u_t = astage.tile([IS, N], FP32, tag="u")
        nc.gpsimd.dma_start(u_t[:], u_rows[:, :])

        eps_col = consts.tile([IS, 1], FP32, tag="epscol")
        nc.vector.memset(eps_col[:], EPS)
        l1 = astage.tile([IS, N], FP32, tag="l1")
        nc.scalar.activation(l1[:], u_t[:], AF.Ln, bias=eps_col[:], scale=1.0)
        l2 = astage.tile([IS, N], FP32, tag="l2")
        # log(eps - l1) = log(-log(u+eps) + eps)
        nc.scalar.activation(l2[:], l1[:], AF.Ln, bias=eps_col[:], scale=-1.0)
        # z = (edge - l2) * (1/tau)
        zt = astage.tile([IS, N], FP32, tag="zt")
        nc.vector.tensor_sub(zt[:], edge_t[:], l2[:])
        z2 = astage.tile([IS, N], FP32, tag="z2")
        nc.vector.tensor_scalar_mul(z2[:], zt[:], 1.0 / TAU)

        mneg = astage.tile([IS, 1], FP32, tag="mneg")
        nc.vector.tensor_reduce(
            mneg[:], z2[:], axis=mybir.AxisListType.X, op=ALU.max, negate=True
        )
        ex = astage.tile([IS, N], FP32, tag="ex")
        zsum = astage.tile([IS, 1], FP32, tag="zsum")
        nc.scalar.activation(
            ex[:], z2[:], AF.Exp, bias=mneg[:], scale=1.0, accum_out=zsum[:]
        )
        zinv = astage.tile([IS, 1], FP32, tag="zinv")
        nc.vector.reciprocal(zinv[:], zsum[:])
        a_f32 = astage.tile([IS, N], FP32, tag="af32")
        nc.vector.tensor_scalar_mul(a_f32[:], ex[:], zinv[:])
        nc.sync.dma_start(a_rows[:, :], a_f32[:])

        a_bf = astage.tile([IS, N], BF16, tag="abf")
        nc.vector.tensor_copy(a_bf[:], a_f32[:])
        a_scr = dram.tile([IS, N], BF16, tag="ascr")
        nc.gpsimd.dma_start(a_scr[:], a_bf[:])

        # ---- per-batch projection chain (PE, fp32) ----
        sb1T = []
        tT = []
        hagg = []
        er = []
        for b in range(B):
            # entity[b] transposed -> eT [d_in, 512]
            eT = work.tile([P, N], FP32, tag="eT")
            for k in range(4):
                ek = work.tile([P, D], FP32, tag="ek")
                nc.sync.dma_start(ek[:], ent[b, k * P : (k + 1) * P, :])
                tp = psum.tile([P, P], FP32, tag="ps_small")
                nc.tensor.transpose(tp[:], ek[:], ident[:])
                nc.vector.tensor_copy(eT[:, k * P : (k + 1) * P], tp[:])

            er_b = perb.tile([IS, D], FP32, tag=f"er{b}")
            nc.sync.dma_start(er_b[:], ent_rows[b, :, :])
            tpr = psum.tile([P, IS], FP32, tag="ps_small")
            nc.tensor.transpose(tpr[:], er_b[:], ident[:IS, :IS])
            eTr = work.tile([P, IS], FP32, tag="eTr")
            nc.vector.tensor_copy(eTr[:], tpr[:])

            pj = psum_big.tile([P, N], FP32, tag="ps_big")
            nc.tensor.matmul(pj[:], Wp_s[:], eT[:], start=True, stop=True)
            projT = work.tile([P, N], FP32, tag="projT")
            nc.scalar.activation(projT[:], pj[:], AF.Identity, bias=bp_col[:])

            pjr = psum.tile([P, IS], FP32, tag="ps_small")
            nc.tensor.matmul(pjr[:], Wp_s[:], eTr[:], start=True, stop=True)
            projTr = work.tile([P, IS], FP32, tag="projTr")
            nc.scalar.activation(projTr[:], pjr[:], AF.Identity, bias=bp_col[:])

            spx = psum_big.tile([P, N], FP32, tag="ps_big")
            nc.tensor.matmul(spx[:], W1s_s[:], projT[:], start=True, stop=True)
            sb1T_b = perb.tile([P, N], BF16, tag=f"sb1_{b}")
            nc.scalar.activation(sb1T_b[:], spx[:], AF.Identity, bias=b1_col[:])

            tpx = psum.tile([P, IS], FP32, tag="ps_small")
            nc.tensor.matmul(tpx[:], W1t_s[:], projTr[:], start=True, stop=True)
            tT_b = perb.tile([P, IS], FP32, tag=f"tT{b}")
            nc.vector.tensor_copy(tT_b[:], tpx[:])

            hagg_b = perb.tile([P, IS], FP32, tag=f"hagg{b}")

            sb1T.append(sb1T_b)
            tT.append(tT_b)
            hagg.append(hagg_b)
            er.append(er_b)

        # ---- main loop: relu + A-weighted aggregation over sources ----
        for i in range(IS):
            arp = arepl_pool.tile([P, N], BF16, tag="arp")
            nc.sync.dma_start(arp[:], a_scr[i : i + 1, :].to_broadcast((P, N)))
            for b in range(B):
                idx = i * B + b
                if _use_act(idx):
                    # ScalarE-assisted path: ACT relu -> DVE 2x bf16 mult ->
                    # ACT copy with fused free-dim accumulation.
                    h = h_pool.tile([P, N], BF16, tag="h")
                    nc.scalar.activation(
                        h[:], sb1T[b][:], AF.Relu, bias=tT[b][:, i : i + 1]
                    )
                    p = h_pool.tile([P, N], BF16, tag="p")
                    nc.vector.tensor_mul(p[:], h[:], arp[:])
                    scr = h_pool.tile([P, N], BF16, tag="scr")
                    nc.scalar.activation(
                        scr[:],
                        p[:],
                        AF.Copy,
                        bias=0.0,
                        accum_out=hagg[b][:, i : i + 1],
                    )
                else:
                    # Fully fused: one VectorE instruction does bias-add,
                    # relu, A-weighting, and the reduction over sources.
                    hw = h_pool.tile([P, N], BF16, tag="hw")
                    nc.vector._custom_dve(
                        rmr_op,
                        out=hw[:],
                        in0=sb1T[b][:],
                        in1=arp[:],
                        s0=tT[b][:, i : i + 1],
                        s1=0.0,
                        accum_out=hagg[b][:, i : i + 1],
                    )

        # ---- tail: upd = hagg @ W2 + b2 ; next = entity_rows + upd ----
        for b in range(B):
            hagg_bf = work.tile([P, IS], BF16, tag="haggbf")
            nc.vector.tensor_copy(hagg_bf[:], hagg[b][:])
            updp = psum.tile([IS, D], FP32, tag="ps_small")
            nc.tensor.matmul(updp[:], hagg_bf[:], W2_bf[:], start=True, stop=False)
            nc.tensor.matmul(updp[:], ones_bf[:], b2_bf[:], start=False, stop=True)
            nextt = work.tile([IS, D], FP32, tag="nextt")
            nc.vector.tensor_add(nextt[:], er[b][:], updp[:])
            nc.sync.dma_start(next_rows[b, :, :], nextt[:])

    # Lower InstISA subclasses (the custom DVE op) to raw instruction bytes;
    # Bacc.compile() does this but plain Bass+Tile does not.
    mybir.codegen_inst_isa_subclasses(nc)

    import types

    orig = type(nc).to_json_bytes
    nc.to_json_bytes = types.MethodType(
        lambda self: _split_multi_waits(orig(self)), nc
    )
    return nc


_CACHE: dict = {}


def kernel(entity_states, Wp, bp, edge_logits, W1, b1, W2, b2, u_noise):
    entity_states = np.ascontiguousarray(entity_states, dtype=np.float32)
    Wp = np.ascontiguousarray(Wp, dtype=np.float32)
    bp = np.ascontiguousarray(bp, dtype=np.float32)
    edge_logits = np.ascontiguousarray(edge_logits, dtype=np.float32)
    W1 = np.ascontiguousarray(W1, dtype=np.float32)
    b1 = np.ascontiguousarray(b1, dtype=np.float32)
    W2 = np.ascontiguousarray(W2, dtype=np.float32)
    b2 = np.ascontiguousarray(b2, dtype=np.float32)
    u_noise = np.ascontiguousarray(u_noise, dtype=np.float32)

    if "nc" not in _CACHE:
        _CACHE["nc"] = build_nc()
    nc = _CACHE["nc"]

    in_maps = []
    for c in range(M):
        sl = slice(c * IS, (c + 1) * IS)
        in_maps.append(
            {
                "entity": entity_states,
                "entity_rows": np.ascontiguousarray(entity_states[:, sl]),
                "edge_rows": np.ascontiguousarray(edge_logits[sl]),
                "u_rows": np.ascontiguousarray(u_noise[sl]),
                "Wp": Wp,
                "bp": bp,
                "W1s": np.ascontiguousarray(W1[:D]),
                "W1t": np.ascontiguousarray(W1[D:]),
                "b1": b1,
                "W2": W2,
                "b2": b2,
            }
        )

    res = run_bass_kernel_spmd(nc, in_maps, list(range(M)))
    results = res.results

    next_state = np.concatenate(
        [results[c]["next_rows"] for c in range(M)], axis=1
    ).astype(np.float32)
    a_full = np.concatenate([results[c]["a_rows"] for c in range(M)], axis=0).astype(
        np.float32
    )
    a_bcast = np.broadcast_to(a_full, (B, N, N)).copy()
    return next_state, a_bcast


# revision 18
# speedup vs baseline: 1.2482x; 1.0306x over previous
"""Trainium2 Bass kernel for nn_CausalSimulationModule (gnn message passing).

next_state = entity + (sum_j A[i,j] * relu(s[j] + t[i] + b1)) @ W2 + b2
A = softmax((edge_logits + gumbel(u)) / tau)   (row-wise over sources j)

Key algebraic move: W2 is linear, so aggregate h over sources j FIRST
(h-agg is (B,N,D)), then apply W2 once -- this removes the (B,N,N,D)@W2
matmul entirely.  The irreducible work is the 134M-element
relu + A-weighted reduction, done as:
  per (i, b):  H[d, j] = relu(sb1T[d, j] + tT[d, i])   (one fused op,
               split between ScalarE (activation bias) and VectorE
               (tensor_scalar add+max, 4x bf16 mode) to balance engines)
               hagg[:, i] = sum_j H * A_repl_i          (one fused DVE
               tensor_tensor_reduce with accum_out)
A_repl_i (row i of A broadcast across the 128 d-partitions) is produced
for free by a stride-0 partition-broadcast DMA from an HBM bf16 scratch.

Sharding: pure target-node sharding. Core c owns i in [c*64, (c+1)*64)
and loops all 4 batches internally. No collectives needed.
"""

import sys

sys.path.insert(0, "/opt/trn_rl_repo")

from contextlib import ExitStack

import numpy as np

import concourse.bass as bass
import concourse.tile as tile
from concourse import mybir
from concourse.masks import make_identity
from concourse.bass_utils import run_bass_kernel_spmd

B, N, D = 4, 512, 128
M = 8            # NeuronCores
IS = N // M      # 64 target rows per core
TAU = 0.5
EPS = 1e-9
P = 128

FP32 = mybir.dt.float32
BF16 = mybir.dt.bfloat16
AF = mybir.ActivationFunctionType
ALU = mybir.AluOpType

# Fraction of the 256 (i, b) units routed through the ScalarE-assisted path
# (ACT relu -> DVE 2x bf16 multiply -> ACT copy+accum); the rest run as one
# fused custom DVE op. Balances ScalarE (~1427ns/unit) vs VectorE saving.
ACT_SHARE_NUM = 104
ACT_SHARE_DEN = 256


def _use_act(idx: int) -> bool:
    # Evenly interleaved split of units between the two paths.
    return (idx * ACT_SHARE_NUM) % ACT_SHARE_DEN < ACT_SHARE_NUM


def _register_relu_mul_reduce():
    """Author a custom fused DVE op:
        out = relu(in0 + s0) * in1 ; accum_out = s1 + sum_free(out)
    This collapses the whole per-(i,b) inner loop (bias-add, relu, A-weight
    multiply, reduction over sources) into ONE VectorE instruction.
    Registered by appending to concourse.dve_ops.OPS at runtime; the uop
    table is generated per-NEFF, so no firmware change is needed."""
    from operator import add as _add

    from concourse import dve_ops as dops
    from concourse.dve_spec import C0, C1, Spec, Src0, Src1, lower, relu
    from concourse.dve_uop import DveOpSpec

    name = "RELU_ADD_MUL_REDUCE_ANT"
    for o in dops.OPS:
        if o.name == name:
            return o

    def _ref(in0, in1, s0, s1, imm2):
        b = (np.maximum(in0.astype(np.float32) + s0, 0) * in1).astype(np.float32)
        return b, s1 + b.reshape(b.shape[0], -1).sum(axis=-1, keepdims=True)

    spec = Spec(body=relu(Src0 + C0) * Src1, accum=_add, accum_init=C1, reference=_ref)
    shas = {}
    for ver in ("v3", "v4"):
        shas[ver] = DveOpSpec(name=name, uops=lower(spec, ver=ver)).sha(ver)
    op = dops.DveOp(name, spec, subdim=False, uops_sha=shas)
    row = dops._CUSTOM_DVE_ROW_BASE + len(dops.OPS)
    dops.OPS.append(op)
    dops._SUB_OPCODE_FOR_NAME[name] = row
    dops.CUSTOM_DVE_SPECS[name] = spec
    return op


def _split_multi_waits(bir_bytes: bytes) -> bytes:
    """Walrus in this container allows at most ONE attached sync wait per
    instruction ("Too many sync wait commands").  Tile attaches several.
    Hoist extra waits onto standalone EventSemaphore instructions (which
    accept up to 2 waits each) inserted just before the instruction on the
    same engine stream.  Waits here are monotonic sem-ge waits, so
    satisfying them sequentially is equivalent to waiting on all at once.
    """
    import json

    bir = json.loads(bir_bytes)
    uid = 0
    for fn in bir["functions"]:
        for blk in fn["blocks"]:
            out = []
            for inst in blk["instructions"]:
                si = inst.get("sync_info")
                waits = (si or {}).get("on_wait") or []
                if len(waits) > 1:
                    extra, keep = waits[:-1], waits[-1:]
                    for k in range(0, len(extra), 1):
                        out.append(
                            {
                                "name": f"{inst['name']}-esw{uid}",
                                "opcode": "EventSemaphore",
                                "engine": inst["engine"],
                                "debug": inst.get("debug", 0),
                                "ins": [],
                                "outs": [],
                                "sync_info": {
                                    "on_update": [],
                                    "on_wait": extra[k : k + 1],
                                },
                            }
                        )
                        uid += 1
                    si["on_wait"] = keep
                out.append(inst)
            blk["instructions"] = out
    return json.dumps(bir).encode()


def build_nc() -> bass.Bass:
    rmr_op = _register_relu_mul_reduce()
    nc = bass.Bass()

    ent = nc.dram_tensor("entity", [B, N, D], FP32, kind="ExternalInput")
    ent_rows = nc.dram_tensor("entity_rows", [B, IS, D], FP32, kind="ExternalInput")
    edge_rows = nc.dram_tensor("edge_rows", [IS, N], FP32, kind="ExternalInput")
    u_rows = nc.dram_tensor("u_rows", [IS, N], FP32, kind="ExternalInput")
    Wp_d = nc.dram_tensor("Wp", [D, D], FP32, kind="ExternalInput")
    bp_d = nc.dram_tensor("bp", [D], FP32, kind="ExternalInput")
    W1s_d = nc.dram_tensor("W1s", [D, D], FP32, kind="ExternalInput")
    W1t_d = nc.dram_tensor("W1t", [D, D], FP32, kind="ExternalInput")
    b1_d = nc.dram_tensor("b1", [D], FP32, kind="ExternalInput")
    W2_d = nc.dram_tensor("W2", [D, D], FP32, kind="ExternalInput")
    b2_d = nc.dram_tensor("b2", [D], FP32, kind="ExternalInput")

    next_rows = nc.dram_tensor("next_rows", [B, IS, D], FP32, kind="ExternalOutput")
    a_rows = nc.dram_tensor("a_rows", [IS, N], FP32, kind="ExternalOutput")

    with tile.TileContext(nc) as tc, ExitStack() as ctx:
        consts = ctx.enter_context(tc.tile_pool(name="consts", bufs=1))
        perb = ctx.enter_context(tc.tile_pool(name="perb", bufs=1))
        work = ctx.enter_context(tc.tile_pool(name="work", bufs=2))
        astage = ctx.enter_context(tc.tile_pool(name="astage", bufs=1))
        arepl_pool = ctx.enter_context(tc.tile_pool(name="arepl", bufs=1))
        h_pool = ctx.enter_context(tc.tile_pool(name="h", bufs=8))
        psum = ctx.enter_context(tc.tile_pool(name="psum", bufs=3, space="PSUM"))
        psum_big = ctx.enter_context(
            tc.tile_pool(name="psum_big", bufs=2, space="PSUM")
        )
        dram = ctx.enter_context(tc.tile_pool(name="dram", bufs=1, space="DRAM"))

        # ---- adjacency inputs first: they head the critical path ----
        edge_t = astage.tile([IS, N], FP32, tag="edge")
        nc.sync.dma_start(edge_t[:], edge_rows[:, :])
        u_t = astage.tile([IS, N], FP32, tag="u")
        nc.sync.dma_start(u_t[:], u_rows[:, :])

        # ---- constants ----
        ident = consts.tile([P, P], FP32, tag="ident")
        make_identity(nc, ident)

        Wp_s = consts.tile([D, D], FP32, tag="Wp")
        nc.gpsimd.dma_start(Wp_s[:], Wp_d[:, :])
        W1s_s = consts.tile([D, D], FP32, tag="W1s")
        nc.gpsimd.dma_start(W1s_s[:], W1s_d[:, :])
        W1t_s = consts.tile([D, D], FP32, tag="W1t")
        nc.gpsimd.dma_start(W1t_s[:], W1t_d[:, :])

        w2f = work.tile([D, D], FP32, tag="w2f")
        nc.gpsimd.dma_start(w2f[:], W2_d[:, :])
        W2_bf = consts.tile([D, D], BF16, tag="W2bf")
        nc.vector.tensor_copy(W2_bf[:], w2f[:])

        bp_col = consts.tile([D, 1], FP32, tag="bp")
        nc.gpsimd.dma_start(bp_col[:], bp_d[:])
        b1_col = consts.tile([D, 1], FP32, tag="b1")
        nc.gpsimd.dma_start(b1_col[:], b1_d[:])

        b2f = work.tile([1, D], FP32, tag="b2f")
        b2_row_ap = bass.AP(tensor=b2_d[:].tensor, offset=0, ap=[[0, 1], [1, D]])
        nc.gpsimd.dma_start(b2f[:], b2_row_ap)
        b2_bf = consts.tile([1, D], BF16, tag="b2bf")
        nc.vector.tensor_copy(b2_bf[:], b2f[:])
        ones_bf = consts.tile([1, IS], BF16, tag="ones")
        nc.vector.memset(ones_bf[:], 1.0)

        # ---- adjacency: A = softmax((edge + gumbel(u)) / tau), fp32 ----
        eps_col = consts.tile([IS, 1], FP32, tag="epscol")
        nc.vector.memset(eps_col[:], EPS)
        l1 = astage.tile([IS, N], FP32, tag="l1")
        nc.scalar.activation(l1[:], u_t[:], AF.Ln, bias=eps_col[:], scale=1.0)
        l2 = astage.tile([IS, N], FP32, tag="l2")
        # log(eps - l1) = log(-log(u+eps) + eps)
        nc.scalar.activation(l2[:], l1[:], AF.Ln, bias=eps_col[:], scale=-1.0)
        # z = (edge - l2) * (1/tau); edge*(1/tau) runs parallel to the Lns
        edge2 = astage.tile([IS, N], FP32, tag="edge2")
        nc.vector.tensor_scalar_mul(edge2[:], edge_t[:], 1.0 / TAU)
        z2 = astage.tile([IS, N], FP32, tag="z2")
        nc.vector.scalar_tensor_tensor(
            z2[:], l2[:], -1.0 / TAU, edge2[:], op0=ALU.mult, op1=ALU.add
        )

        mneg = astage.tile([IS, 1], FP32, tag="mneg")
        nc.vector.tensor_reduce(
            mneg[:], z2[:], axis=mybir.AxisListType.X, op=ALU.max, negate=True
        )
        ex = astage.tile([IS, N], FP32, tag="ex")
        zsum = astage.tile([IS, 1], FP32, tag="zsum")
        nc.scalar.activation(
            ex[:], z2[:], AF.Exp, bias=mneg[:], scale=1.0, accum_out=zsum[:]
        )
        zinv = astage.tile([IS, 1], FP32, tag="zinv")
        nc.vector.reciprocal(zinv[:], zsum[:])
        a_f32 = astage.tile([IS, N], FP32, tag="af32")
        nc.vector.tensor_scalar_mul(a_f32[:], ex[:], zinv[:])
        nc.sync.dma_start(a_rows[:, :], a_f32[:])

        a_bf = astage.tile([IS, N], BF16, tag="abf")
        nc.vector.tensor_copy(a_bf[:], a_f32[:])
        a_scr = dram.tile([IS, N], BF16, tag="ascr")
        nc.gpsimd.dma_start(a_scr[:], a_bf[:])

        # ---- preload all A_repl broadcast tiles (row i of A replicated
        # across the 128 d-partitions), resident for the whole kernel ----
        arps = []
        for i in range(IS):
            arp = arepl_pool.tile([P, N], BF16, tag=f"arp{i}")
            nc.sync.dma_start(arp[:], a_scr[i : i + 1, :].to_broadcast((P, N)))
            arps.append(arp)

        # ---- per-batch: projection chain (PE), main loop, tail ----
        # b is the OUTER loop so batch b's units overlap batch b+1's
        # projection chain and batch b-1's tail.
        for b in range(B):
            # entity[b] transposed -> eT [d_in, 512]
            eT = work.tile([P, N], FP32, tag="eT")
            for k in range(4):
                ek = work.tile([P, D], FP32, tag="ek")
                nc.sync.dma_start(ek[:], ent[b, k * P : (k + 1) * P, :])
                tp = psum.tile([P, P], FP32, tag="ps_small")
                nc.tensor.transpose(tp[:], ek[:], ident[:])
                nc.vector.tensor_copy(eT[:, k * P : (k + 1) * P], tp[:])

            er_b = perb.tile([IS, D], FP32, tag=f"er{b}")
            nc.sync.dma_start(er_b[:], ent_rows[b, :, :])
            tpr = psum.tile([P, IS], FP32, tag="ps_small")
            nc.tensor.transpose(tpr[:], er_b[:], ident[:IS, :IS])
            eTr = work.tile([P, IS], FP32, tag="eTr")
            nc.vector.tensor_copy(eTr[:], tpr[:])

            pj = psum_big.tile([P, N], FP32, tag="ps_big")
            nc.tensor.matmul(pj[:], Wp_s[:], eT[:], start=True, stop=True)
            projT = work.tile([P, N], FP32, tag="projT")
            nc.scalar.activation(projT[:], pj[:], AF.Identity, bias=bp_col[:])

            pjr = psum.tile([P, IS], FP32, tag="ps_small")
            nc.tensor.matmul(pjr[:], Wp_s[:], eTr[:], start=True, stop=True)
            projTr = work.tile([P, IS], FP32, tag="projTr")
            nc.scalar.activation(projTr[:], pjr[:], AF.Identity, bias=bp_col[:])

            spx = psum_big.tile([P, N], FP32, tag="ps_big")
            nc.tensor.matmul(spx[:], W1s_s[:], projT[:], start=True, stop=True)
            sb1T_b = perb.tile([P, N], BF16, tag=f"sb1_{b}")
            nc.scalar.activation(sb1T_b[:], spx[:], AF.Identity, bias=b1_col[:])

            tpx = psum.tile([P, IS], FP32, tag="ps_small")
            nc.tensor.matmul(tpx[:], W1t_s[:], projTr[:], start=True, stop=True)
            tT_b = perb.tile([P, IS], FP32, tag=f"tT{b}")
            nc.vector.tensor_copy(tT_b[:], tpx[:])

            hagg_b = perb.tile([P, IS], FP32, tag=f"hagg{b}")

            # main loop: relu + A-weighted aggregation over sources
            for i in range(IS):
                idx = b * IS + i
                arp = arps[i]
                if _use_act(idx):
                    # ScalarE-assisted path: ACT relu -> DVE 2x bf16 mult ->
                    # ACT copy with fused free-dim accumulation.
                    h = h_pool.tile([P, N], BF16, tag="h")
                    nc.scalar.activation(
                        h[:], sb1T_b[:], AF.Relu, bias=tT_b[:, i : i + 1]
                    )
                    p = h_pool.tile([P, N], BF16, tag="p")
                    nc.vector.tensor_mul(p[:], h[:], arp[:])
                    scr = h_pool.tile([P, N], BF16, tag="scr")
                    nc.scalar.activation(
                        scr[:],
                        p[:],
                        AF.Copy,
                        bias=0.0,
                        accum_out=hagg_b[:, i : i + 1],
                    )
                else:
                    # Fully fused: one VectorE instruction does bias-add,
                    # relu, A-weighting, and the reduction over sources.
                    hw = h_pool.tile([P, N], BF16, tag="hw")
                    nc.vector._custom_dve(
                        rmr_op,
                        out=hw[:],
                        in0=sb1T_b[:],
                        in1=arp[:],
                        s0=tT_b[:, i : i + 1],
                        s1=0.0,
                        accum_out=hagg_b[:, i : i + 1],
                    )

            # tail: upd = hagg @ W2 + b2 ; next = entity_rows + upd
            hagg_bf = work.tile([P, IS], BF16, tag="haggbf")
            nc.vector.tensor_copy(hagg_bf[:], hagg_b[:])
            updp = psum.tile([IS, D], FP32, tag="ps_small")
            nc.tensor.matmul(updp[:], hagg_bf[:], W2_bf[:], start=True, stop=False)
            nc.tensor.matmul(updp[:], ones_bf[:], b2_bf[:], start=False, stop=True)
            nextt = work.tile([IS, D], FP32, tag="nextt")
            nc.vector.tensor_add(nextt[:], er_b[:], updp[:])
            nc.sync.dma_start(next_rows[b, :, :], nextt[:])

    # Lower InstISA subclasses (the custom DVE op) to raw instruction bytes;
    # Bacc.compile() does this but plain Bass+Tile does not.
    mybir.codegen_inst_isa_subclasses(nc)

    import types

    orig = type(nc).to_json_bytes
    nc.to_json_bytes = types.MethodType(
        lambda self: _split_multi_waits(orig(self)), nc
    )
    return nc


_CACHE: dict = {}


def kernel(entity_states, Wp, bp, edge_logits, W1, b1, W2, b2, u_noise):
    entity_states = np.ascontiguousarray(entity_states, dtype=np.float32)
    Wp = np.ascontiguousarray(Wp, dtype=np.float32)
    bp = np.ascontiguousarray(bp, dtype=np.float32)
    edge_logits = np.ascontiguousarray(edge_logits, dtype=np.float32)
    W1 = np.ascontiguousarray(W1, dtype=np.float32)
    b1 = np.ascontiguousarray(b1, dtype=np.float32)
    W2 = np.ascontiguousarray(W2, dtype=np.float32)
    b2 = np.ascontiguousarray(b2, dtype=np.float32)
    u_noise = np.ascontiguousarray(u_noise, dtype=np.float32)

    if "nc" not in _CACHE:
        _CACHE["nc"] = build_nc()
    nc = _CACHE["nc"]

    in_maps = []
    for c in range(M):
        sl = slice(c * IS, (c + 1) * IS)
        in_maps.append(
            {
                "entity": entity_states,
                "entity_rows": np.ascontiguousarray(entity_states[:, sl]),
                "edge_rows": np.ascontiguousarray(edge_logits[sl]),
                "u_rows": np.ascontiguousarray(u_noise[sl]),
                "Wp": Wp,
                "bp": bp,
                "W1s": np.ascontiguousarray(W1[:D]),
                "W1t": np.ascontiguousarray(W1[D:]),
                "b1": b1,
                "W2": W2,
                "b2": b2,
            }
        )

    res = run_bass_kernel_spmd(nc, in_maps, list(range(M)))
    results = res.results

    next_state = np.concatenate(
        [results[c]["next_rows"] for c in range(M)], axis=1
    ).astype(np.float32)
    a_full = np.concatenate([results[c]["a_rows"] for c in range(M)], axis=0).astype(
        np.float32
    )
    a_bcast = np.broadcast_to(a_full, (B, N, N)).copy()
    return next_state, a_bcast


# revision 22
# speedup vs baseline: 1.3253x; 1.0617x over previous
"""Trainium2 Bass kernel for nn_CausalSimulationModule (gnn message passing).

next_state = entity + (sum_j A[i,j] * relu(s[j] + t[i] + b1)) @ W2 + b2
A = softmax((edge_logits + gumbel(u)) / tau)   (row-wise over sources j)

Key algebraic move: W2 is linear, so aggregate h over sources j FIRST
(h-agg is (B,N,D)), then apply W2 once -- this removes the (B,N,N,D)@W2
matmul entirely.  The irreducible work is the 134M-element
relu + A-weighted reduction, done as:
  per (i, b):  H[d, j] = relu(sb1T[d, j] + tT[d, i])   (one fused op,
               split between ScalarE (activation bias) and VectorE
               (tensor_scalar add+max, 4x bf16 mode) to balance engines)
               hagg[:, i] = sum_j H * A_repl_i          (one fused DVE
               tensor_tensor_reduce with accum_out)
A_repl_i (row i of A broadcast across the 128 d-partitions) is produced
for free by a stride-0 partition-broadcast DMA from an HBM bf16 scratch.

Sharding: pure target-node sharding. Core c owns i in [c*64, (c+1)*64)
and loops all 4 batches internally. No collectives needed.
"""

import os
import sys

sys.path.insert(0, "/opt/trn_rl_repo")

from contextlib import ExitStack

import numpy as np

import concourse.bass as bass
import concourse.tile as tile
from concourse import mybir
from concourse.masks import make_identity
from concourse.bass_utils import run_bass_kernel_spmd

B, N, D = 4, 512, 128
M = 8            # NeuronCores
IS = N // M      # 64 target rows per core
TAU = 0.5
EPS = 1e-9
P = 128

FP32 = mybir.dt.float32
BF16 = mybir.dt.bfloat16
AF = mybir.ActivationFunctionType
ALU = mybir.AluOpType

# Fraction of the 256 (i, b) units routed through the ScalarE-assisted path
# (ACT relu -> DVE 2x bf16 multiply -> ACT copy+accum); the rest run as one
# fused custom DVE op. Balances ScalarE (~1427ns/unit) vs VectorE saving.
ACT_SHARE_NUM = 101
ACT_SHARE_DEN = 256


def _use_act(idx: int) -> bool:
    # Evenly interleaved split of units between the two paths.
    return (idx * ACT_SHARE_NUM) % ACT_SHARE_DEN < ACT_SHARE_NUM


def _register_relu_mul_reduce():
    """Author a custom fused DVE op:
        out = relu(in0 + s0) * in1 ; accum_out = s1 + sum_free(out)
    This collapses the whole per-(i,b) inner loop (bias-add, relu, A-weight
    multiply, reduction over sources) into ONE VectorE instruction.
    Registered by appending to concourse.dve_ops.OPS at runtime; the uop
    table is generated per-NEFF, so no firmware change is needed."""
    from operator import add as _add

    from concourse import dve_ops as dops
    from concourse.dve_spec import C0, C1, Spec, Src0, Src1, lower, relu
    from concourse.dve_uop import DveOpSpec

    name = "RELU_ADD_MUL_REDUCE_ANT"
    for o in dops.OPS:
        if o.name == name:
            return o

    def _ref(in0, in1, s0, s1, imm2):
        b = (np.maximum(in0.astype(np.float32) + s0, 0) * in1).astype(np.float32)
        return b, s1 + b.reshape(b.shape[0], -1).sum(axis=-1, keepdims=True)

    spec = Spec(body=relu(Src0 + C0) * Src1, accum=_add, accum_init=C1, reference=_ref)
    shas = {}
    for ver in ("v3", "v4"):
        shas[ver] = DveOpSpec(name=name, uops=lower(spec, ver=ver)).sha(ver)
    op = dops.DveOp(name, spec, subdim=False, uops_sha=shas)
    row = dops._CUSTOM_DVE_ROW_BASE + len(dops.OPS)
    dops.OPS.append(op)
    dops._SUB_OPCODE_FOR_NAME[name] = row
    dops.CUSTOM_DVE_SPECS[name] = spec

    # ---- hand-written 2X_1PORT program ----------------------------------
    # bf16 packed mode: each 32-bit port read carries two elements; the
    # crossbar exposes them as SRC_* (lo) and SRC_*_HI (hi). Stages 0-2
    # compute relu(in0_lo+s0)*in1_lo, stages 3-5 the hi half, stage 6 adds
    # the pair, stage 7 accumulates into CURR_ALU_OUT. Both products are
    # emitted packed via WR0_LO/WR0_HI. Exactly fills the 8-stage budget.
    from concourse.dve_uop import (
        ENABLE,
        AluInp,
        AluOp,
        DelayInp,
        InpSel,
        OutPath,
        OutSel,
        Trigger,
        UopConfig,
        UopDpConfig,
    )

    def _dp():
        return UopDpConfig()

    # seed uop: route accum_init (CONST_1) down delay lane 0, latch it into
    # stage 7's CURR_ALU_OUT; consumes no stream data, writes nothing.
    seed = UopConfig()
    seed.enable_input(InpSel.CONST_1, 1)
    for s in range(7):
        seed.datapath_config[s] = _dp().pass_through_alu().pass_through_delay(0)
    seed.datapath_config[7] = _dp().enable_alu(AluOp.BYPASS, AluInp.PREV_DELAY_0)
    seed.datapath_config[7].alu_out_a_enable = ENABLE
    seed.repeat_count = 1
    seed.trigger = (Trigger.COUNT, Trigger.NONE, Trigger.NONE)
    seed.next_uop = (1, 0, 0)

    st = UopConfig()
    st.enable_input(InpSel.SRC_0, 1)      # d0 = in0_lo
    st.enable_input(InpSel.SRC_0_HI, 2)   # d1 = in0_hi
    st.enable_input(InpSel.CONST_0, 3)    # d2 = s0
    st.enable_input(InpSel.ZERO, 4)       # d3 = 0
    st.enable_input(InpSel.SRC_1, 5)      # d4 = in1_lo
    st.enable_input(InpSel.SRC_1_HI, 6)   # d5 = in1_hi
    st.datapath_config[0] = (
        _dp()
        .enable_alu(AluOp.ADD, AluInp.PREV_DELAY_0, AluInp.PREV_DELAY_2)
        .pass_through_delay(1, 2, 3, 4, 5)
    )
    st.datapath_config[1] = (
        _dp()
        .enable_alu(AluOp.MAX, AluInp.PREV_ALU_OUT, AluInp.PREV_DELAY_3)
        .pass_through_delay(1, 2, 3, 4, 5)
    )
    st.datapath_config[2] = (
        _dp()
        .enable_alu(AluOp.MULTIPLY, AluInp.PREV_ALU_OUT, AluInp.PREV_DELAY_4)
        .pass_through_delay(1, 2, 3, 5)
    )
    st.datapath_config[3] = (
        _dp()
        .enable_alu(AluOp.ADD, AluInp.PREV_DELAY_1, AluInp.PREV_DELAY_2)
        .enable_delay_from_src(DelayInp.PREV_ALU_OUT, 0)  # capture p_lo
        .pass_through_delay(3, 5)
    )
    st.datapath_config[4] = (
        _dp()
        .enable_alu(AluOp.MAX, AluInp.PREV_ALU_OUT, AluInp.PREV_DELAY_3)
        .pass_through_delay(0, 5)
    )
    st.datapath_config[5] = (
        _dp()
        .enable_alu(AluOp.MULTIPLY, AluInp.PREV_ALU_OUT, AluInp.PREV_DELAY_5)
        .pass_through_delay(0)
    )
    st.datapath_config[6] = (
        _dp()
        .enable_alu(AluOp.ADD, AluInp.PREV_ALU_OUT, AluInp.PREV_DELAY_0)
        .enable_delay_from_src(DelayInp.PREV_ALU_OUT, 1)  # capture p_hi
        .pass_through_delay(0)
    )
    st.datapath_config[7] = (
        _dp()
        .enable_alu(AluOp.ADD, AluInp.CURR_ALU_OUT, AluInp.PREV_ALU_OUT)
        .pass_through_delay(0, 1)
    )
    st.datapath_config[7].alu_out_a_enable = ENABLE
    st.require_inp0 = ENABLE
    st.require_inp1 = ENABLE
    st.trigger = (Trigger.SRC_TENSOR_DONE, Trigger.NONE, Trigger.NONE)
    st.enable_output(OutSel.DELAY_0, OutPath.WR0_LO)
    st.enable_output(OutSel.DELAY_1, OutPath.WR0_HI)

    spec2x = DveOpSpec(
        name=name,
        opcode=row,
        uops=lower(spec, ver="v3"),
        uops_2x=[seed, st],
        perf_max=1,
        rd1_en=True,
    )
    spec2x.validate("v3")
    if os.environ.get("RMR_2X"):
        dops._COMPILE_CACHE[(name, "v3")] = spec2x
    return op


def _split_multi_waits(bir_bytes: bytes) -> bytes:
    """Walrus in this container allows at most ONE attached sync wait per
    instruction ("Too many sync wait commands").  Tile attaches several.
    Hoist extra waits onto standalone EventSemaphore instructions (which
    accept up to 2 waits each) inserted just before the instruction on the
    same engine stream.  Waits here are monotonic sem-ge waits, so
    satisfying them sequentially is equivalent to waiting on all at once.
    """
    import json

    bir = json.loads(bir_bytes)
    uid = 0
    for fn in bir["functions"]:
        for blk in fn["blocks"]:
            out = []
            for inst in blk["instructions"]:
                si = inst.get("sync_info")
                waits = (si or {}).get("on_wait") or []
                if len(waits) > 1:
                    extra, keep = waits[:-1], waits[-1:]
                    for k in range(0, len(extra), 1):
                        out.append(
                            {
                                "name": f"{inst['name']}-esw{uid}",
                                "opcode": "EventSemaphore",
                                "engine": inst["engine"],
                                "debug": inst.get("debug", 0),
                                "ins": [],
                                "outs": [],
                                "sync_info": {
                                    "on_update": [],
                                    "on_wait": extra[k : k + 1],
                                },
                            }
                        )
                        uid += 1
                    si["on_wait"] = keep
                out.append(inst)
            blk["instructions"] = out
    return json.dumps(bir).encode()


def build_nc() -> bass.Bass:
    rmr_op = _register_relu_mul_reduce()
    nc = bass.Bass()

    ent = nc.dram_tensor("entity", [B, N, D], FP32, kind="ExternalInput")
    ent_rows = nc.dram_tensor("entity_rows", [B, IS, D], FP32, kind="ExternalInput")
    edge_rows = nc.dram_tensor("edge_rows", [IS, N], FP32, kind="ExternalInput")
    u_rows = nc.dram_tensor("u_rows", [IS, N], FP32, kind="ExternalInput")
    Wp_d = nc.dram_tensor("Wp", [D, D], FP32, kind="ExternalInput")
    bp_d = nc.dram_tensor("bp", [D], FP32, kind="ExternalInput")
    W1s_d = nc.dram_tensor("W1s", [D, D], FP32, kind="ExternalInput")
    W1t_d = nc.dram_tensor("W1t", [D, D], FP32, kind="ExternalInput")
    b1_d = nc.dram_tensor("b1", [D], FP32, kind="ExternalInput")
    W2_d = nc.dram_tensor("W2", [D, D], FP32, kind="ExternalInput")
    b2_d = nc.dram_tensor("b2", [D], FP32, kind="ExternalInput")

    next_rows = nc.dram_tensor("next_rows", [B, IS, D], FP32, kind="ExternalOutput")
    a_rows = nc.dram_tensor("a_rows", [IS, N], FP32, kind="ExternalOutput")

    with tile.TileContext(nc) as tc, ExitStack() as ctx:
        consts = ctx.enter_context(tc.tile_pool(name="consts", bufs=1))
        perb = ctx.enter_context(tc.tile_pool(name="perb", bufs=1))
        work = ctx.enter_context(tc.tile_pool(name="work", bufs=3))
        astage = ctx.enter_context(tc.tile_pool(name="astage", bufs=1))
        arepl_pool = ctx.enter_context(tc.tile_pool(name="arepl", bufs=1))
        h_pool = ctx.enter_context(tc.tile_pool(name="h", bufs=8))
        psum = ctx.enter_context(tc.tile_pool(name="psum", bufs=3, space="PSUM"))
        psum_big = ctx.enter_context(
            tc.tile_pool(name="psum_big", bufs=2, space="PSUM")
        )
        dram = ctx.enter_context(tc.tile_pool(name="dram", bufs=1, space="DRAM"))

        # ---- adjacency inputs first: they head the critical path ----
        edge_t = astage.tile([IS, N], FP32, tag="edge")
        nc.sync.dma_start(edge_t[:], edge_rows[:, :])
        u_t = astage.tile([IS, N], FP32, tag="u")
        nc.sync.dma_start(u_t[:], u_rows[:, :])

        # ---- constants ----
        ident = consts.tile([P, P], FP32, tag="ident")
        make_identity(nc, ident)

        Wp_s = consts.tile([D, D], FP32, tag="Wp")
        nc.gpsimd.dma_start(Wp_s[:], Wp_d[:, :])
        W1s_s = consts.tile([D, D], FP32, tag="W1s")
        nc.gpsimd.dma_start(W1s_s[:], W1s_d[:, :])
        W1t_s = consts.tile([D, D], FP32, tag="W1t")
        nc.gpsimd.dma_start(W1t_s[:], W1t_d[:, :])

        w2f = work.tile([D, D], FP32, tag="w2f")
        nc.gpsimd.dma_start(w2f[:], W2_d[:, :])
        W2_bf = consts.tile([D, D], BF16, tag="W2bf")
        nc.vector.tensor_copy(W2_bf[:], w2f[:])

        bp_col = consts.tile([D, 1], FP32, tag="bp")
        nc.gpsimd.dma_start(bp_col[:], bp_d[:])
        b1_col = consts.tile([D, 1], FP32, tag="b1")
        nc.gpsimd.dma_start(b1_col[:], b1_d[:])

        b2f = work.tile([1, D], FP32, tag="b2f")
        b2_row_ap = bass.AP(tensor=b2_d[:].tensor, offset=0, ap=[[0, 1], [1, D]])
        nc.gpsimd.dma_start(b2f[:], b2_row_ap)
        b2_bf = consts.tile([1, D], BF16, tag="b2bf")
        nc.vector.tensor_copy(b2_bf[:], b2f[:])
        ones_bf = consts.tile([1, IS], BF16, tag="ones")
        nc.vector.memset(ones_bf[:], 1.0)

        # ---- adjacency: A = softmax((edge + gumbel(u)) / tau), fp32 ----
        eps_col = consts.tile([IS, 1], FP32, tag="epscol")
        nc.vector.memset(eps_col[:], EPS)
        l1 = astage.tile([IS, N], FP32, tag="l1")
        nc.scalar.activation(l1[:], u_t[:], AF.Ln, bias=eps_col[:], scale=1.0)
        l2 = astage.tile([IS, N], FP32, tag="l2")
        # log(eps - l1) = log(-log(u+eps) + eps)
        nc.scalar.activation(l2[:], l1[:], AF.Ln, bias=eps_col[:], scale=-1.0)
        # z = (edge - l2) * (1/tau); edge*(1/tau) runs parallel to the Lns
        edge2 = astage.tile([IS, N], FP32, tag="edge2")
        nc.vector.tensor_scalar_mul(edge2[:], edge_t[:], 1.0 / TAU)
        z2 = astage.tile([IS, N], FP32, tag="z2")
        nc.vector.scalar_tensor_tensor(
            z2[:], l2[:], -1.0 / TAU, edge2[:], op0=ALU.mult, op1=ALU.add
        )

        mneg = astage.tile([IS, 1], FP32, tag="mneg")
        nc.vector.tensor_reduce(
            mneg[:], z2[:], axis=mybir.AxisListType.X, op=ALU.max, negate=True
        )
        ex = astage.tile([IS, N], FP32, tag="ex")
        zsum = astage.tile([IS, 1], FP32, tag="zsum")
        nc.scalar.activation(
            ex[:], z2[:], AF.Exp, bias=mneg[:], scale=1.0, accum_out=zsum[:]
        )
        zinv = astage.tile([IS, 1], FP32, tag="zinv")
        nc.vector.reciprocal(zinv[:], zsum[:])
        a_f32 = astage.tile([IS, N], FP32, tag="af32")
        nc.vector.tensor_scalar_mul(a_f32[:], ex[:], zinv[:])
        nc.sync.dma_start(a_rows[:, :], a_f32[:])

        a_bf = astage.tile([IS, N], BF16, tag="abf")
        nc.vector.tensor_copy(a_bf[:], a_f32[:])
        a_scr = dram.tile([IS, N], BF16, tag="ascr")
        nc.gpsimd.dma_start(a_scr[:], a_bf[:])

        # ---- preload all A_repl broadcast tiles (row i of A replicated
        # across the 128 d-partitions), resident for the whole kernel ----
        arps = []
        for i in range(IS):
            arp = arepl_pool.tile([P, N], BF16, tag=f"arp{i}")
            nc.sync.dma_start(arp[:], a_scr[i : i + 1, :].to_broadcast((P, N)))
            arps.append(arp)

        # ---- per-batch: projection chain (PE), main loop, tail ----
        # b is the OUTER loop so batch b's units overlap batch b+1's
        # projection chain and batch b-1's tail.
        for b in range(B):
            # entity[b] transposed -> eT [d_in, 512]
            eT = work.tile([P, N], FP32, tag="eT")
            for k in range(4):
                ek = work.tile([P, D], FP32, tag="ek")
                nc.sync.dma_start(ek[:], ent[b, k * P : (k + 1) * P, :])
                tp = psum.tile([P, P], FP32, tag="ps_small")
                nc.tensor.transpose(tp[:], ek[:], ident[:])
                nc.vector.tensor_copy(eT[:, k * P : (k + 1) * P], tp[:])

            er_b = perb.tile([IS, D], FP32, tag=f"er{b}")
            nc.sync.dma_start(er_b[:], ent_rows[b, :, :])
            tpr = psum.tile([P, IS], FP32, tag="ps_small")
            nc.tensor.transpose(tpr[:], er_b[:], ident[:IS, :IS])
            eTr = work.tile([P, IS], FP32, tag="eTr")
            nc.vector.tensor_copy(eTr[:], tpr[:])

            pj = psum_big.tile([P, N], FP32, tag="ps_big")
            nc.tensor.matmul(pj[:], Wp_s[:], eT[:], start=True, stop=True)
            projT = work.tile([P, N], FP32, tag="projT")
            nc.scalar.activation(projT[:], pj[:], AF.Identity, bias=bp_col[:])

            pjr = psum.tile([P, IS], FP32, tag="ps_small")
            nc.tensor.matmul(pjr[:], Wp_s[:], eTr[:], start=True, stop=True)
            projTr = work.tile([P, IS], FP32, tag="projTr")
            nc.scalar.activation(projTr[:], pjr[:], AF.Identity, bias=bp_col[:])

            spx = psum_big.tile([P, N], FP32, tag="ps_big")
            nc.tensor.matmul(spx[:], W1s_s[:], projT[:], start=True, stop=True)
            sb1T_b = perb.tile([P, N], BF16, tag=f"sb1_{b}")
            nc.scalar.activation(sb1T_b[:], spx[:], AF.Identity, bias=b1_col[:])

            tpx = psum.tile([P, IS], FP32, tag="ps_small")
            nc.tensor.matmul(tpx[:], W1t_s[:], projTr[:], start=True, stop=True)
            tT_b = perb.tile([P, IS], FP32, tag=f"tT{b}")
            nc.vector.tensor_copy(tT_b[:], tpx[:])

            hagg_b = perb.tile([P, IS], FP32, tag=f"hagg{b}")

            # main loop: relu + A-weighted aggregation over sources
            for i in range(IS):
                idx = b * IS + i
                arp = arps[i]
                if _use_act(idx):
                    # ScalarE-assisted path: ACT relu -> DVE 2x bf16 mult ->
                    # ACT copy with fused free-dim accumulation.
                    h = h_pool.tile([P, N], BF16, tag="h")
                    nc.scalar.activation(
                        h[:], sb1T_b[:], AF.Relu, bias=tT_b[:, i : i + 1]
                    )
                    p = h_pool.tile([P, N], BF16, tag="p")
                    nc.vector.tensor_mul(p[:], h[:], arp[:])
                    scr = h_pool.tile([P, N], BF16, tag="scr")
                    nc.scalar.activation(
                        scr[:],
                        p[:],
                        AF.Copy,
                        bias=0.0,
                        accum_out=hagg_b[:, i : i + 1],
                    )
                else:
                    # Fully fused: one VectorE instruction does bias-add,
                    # relu, A-weighting, and the reduction over sources.
                    hw = h_pool.tile([P, N], BF16, tag="hw")
                    ci = nc.vector._custom_dve(
                        rmr_op,
                        out=hw[:],
                        in0=sb1T_b[:],
                        in1=arp[:],
                        s0=tT_b[:, i : i + 1],
                        s1=0.0,
                        accum_out=hagg_b[:, i : i + 1],
                    )
                    # perf_max left at 0: run the verified REGULAR program
                    _ = ci

            # tail: upd = hagg @ W2 + b2 ; next = entity_rows + upd
            hagg_bf = work.tile([P, IS], BF16, tag="haggbf")
            nc.vector.tensor_copy(hagg_bf[:], hagg_b[:])
            updp = psum.tile([IS, D], FP32, tag="ps_small")
            nc.tensor.matmul(updp[:], hagg_bf[:], W2_bf[:], start=True, stop=False)
            nc.tensor.matmul(updp[:], ones_bf[:], b2_bf[:], start=False, stop=True)
            nextt = work.tile([IS, D], FP32, tag="nextt")
            nc.vector.tensor_add(nextt[:], er_b[:], updp[:])
            nc.sync.dma_start(next_rows[b, :, :], nextt[:])

    # Lower InstISA subclasses (the custom DVE op) to raw instruction bytes;
    # Bacc.compile() does this but plain Bass+Tile does not.
    mybir.codegen_inst_isa_subclasses(nc)

    import types

    orig = type(nc).to_json_bytes
    nc.to_json_bytes = types.MethodType(
        lambda self: _split_multi_waits(orig(self)), nc
    )
    return nc


_CACHE: dict = {}


def kernel(entity_states, Wp, bp, edge_logits, W1, b1, W2, b2, u_noise):
    entity_states = np.ascontiguousarray(entity_states, dtype=np.float32)
    Wp = np.ascontiguousarray(Wp, dtype=np.float32)
    bp = np.ascontiguousarray(bp, dtype=np.float32)
    edge_logits = np.ascontiguousarray(edge_logits, dtype=np.float32)
    W1 = np.ascontiguousarray(W1, dtype=np.float32)
    b1 = np.ascontiguousarray(b1, dtype=np.float32)
    W2 = np.ascontiguousarray(W2, dtype=np.float32)
    b2 = np.ascontiguousarray(b2, dtype=np.float32)
    u_noise = np.ascontiguousarray(u_noise, dtype=np.float32)

    if "nc" not in _CACHE:
        _CACHE["nc"] = build_nc()
    nc = _CACHE["nc"]

    in_maps = []
    for c in range(M):
        sl = slice(c * IS, (c + 1) * IS)
        in_maps.append(
            {
                "entity": entity_states,
                "entity_rows": np.ascontiguousarray(entity_states[:, sl]),
                "edge_rows": np.ascontiguousarray(edge_logits[sl]),
                "u_rows": np.ascontiguousarray(u_noise[sl]),
                "Wp": Wp,
                "bp": bp,
                "W1s": np.ascontiguousarray(W1[:D]),
                "W1t": np.ascontiguousarray(W1[D:]),
                "b1": b1,
                "W2": W2,
                "b2": b2,
            }
        )

    res = run_bass_kernel_spmd(nc, in_maps, list(range(M)))
    results = res.results

    next_state = np.concatenate(
        [results[c]["next_rows"] for c in range(M)], axis=1
    ).astype(np.float32)
    a_full = np.concatenate([results[c]["a_rows"] for c in range(M)], axis=0).astype(
        np.float32
    )
    a_bcast = np.broadcast_to(a_full, (B, N, N)).copy()
    return next_state, a_bcast


# revision 23
# speedup vs baseline: 1.3507x; 1.0192x over previous
"""Trainium2 Bass kernel for nn_CausalSimulationModule (gnn message passing).

next_state = entity + (sum_j A[i,j] * relu(s[j] + t[i] + b1)) @ W2 + b2
A = softmax((edge_logits + gumbel(u)) / tau)   (row-wise over sources j)

Key algebraic move: W2 is linear, so aggregate h over sources j FIRST
(h-agg is (B,N,D)), then apply W2 once -- this removes the (B,N,N,D)@W2
matmul entirely.  The irreducible work is the 134M-element
relu + A-weighted reduction, done as:
  per (i, b):  H[d, j] = relu(sb1T[d, j] + tT[d, i])   (one fused op,
               split between ScalarE (activation bias) and VectorE
               (tensor_scalar add+max, 4x bf16 mode) to balance engines)
               hagg[:, i] = sum_j H * A_repl_i          (one fused DVE
               tensor_tensor_reduce with accum_out)
A_repl_i (row i of A broadcast across the 128 d-partitions) is produced
for free by a stride-0 partition-broadcast DMA from an HBM bf16 scratch.

Sharding: pure target-node sharding. Core c owns i in [c*64, (c+1)*64)
and loops all 4 batches internally. No collectives needed.
"""

import os
import sys

sys.path.insert(0, "/opt/trn_rl_repo")

from contextlib import ExitStack

import numpy as np

import concourse.bass as bass
import concourse.tile as tile
from concourse import mybir
from concourse.masks import make_identity
from concourse.bass_utils import run_bass_kernel_spmd

B, N, D = 4, 512, 128
M = 8            # NeuronCores
IS = N // M      # 64 target rows per core
TAU = 0.5
EPS = 1e-9
P = 128

FP32 = mybir.dt.float32
BF16 = mybir.dt.bfloat16
AF = mybir.ActivationFunctionType
ALU = mybir.AluOpType

# Fraction of the 256 (i, b) units routed through the ScalarE-assisted path
# (ACT relu -> DVE 2x bf16 multiply -> ACT copy+accum); the rest run as one
# fused custom DVE op. Balances ScalarE (~1427ns/unit) vs VectorE saving.
ACT_SHARE_NUM = 97
ACT_SHARE_DEN = 256


def _use_act(idx: int) -> bool:
    # Evenly interleaved split of units between the two paths.
    return (idx * ACT_SHARE_NUM) % ACT_SHARE_DEN < ACT_SHARE_NUM


def _register_relu_mul_reduce():
    """Author a custom fused DVE op:
        out = relu(in0 + s0) * in1 ; accum_out = s1 + sum_free(out)
    This collapses the whole per-(i,b) inner loop (bias-add, relu, A-weight
    multiply, reduction over sources) into ONE VectorE instruction.
    Registered by appending to concourse.dve_ops.OPS at runtime; the uop
    table is generated per-NEFF, so no firmware change is needed."""
    from operator import add as _add

    from concourse import dve_ops as dops
    from concourse.dve_spec import C0, C1, Spec, Src0, Src1, lower, relu
    from concourse.dve_uop import DveOpSpec

    name = "RELU_ADD_MUL_REDUCE_ANT"
    for o in dops.OPS:
        if o.name == name:
            return o

    def _ref(in0, in1, s0, s1, imm2):
        b = (np.maximum(in0.astype(np.float32) + s0, 0) * in1).astype(np.float32)
        return b, s1 + b.reshape(b.shape[0], -1).sum(axis=-1, keepdims=True)

    spec = Spec(body=relu(Src0 + C0) * Src1, accum=_add, accum_init=C1, reference=_ref)
    shas = {}
    for ver in ("v3", "v4"):
        shas[ver] = DveOpSpec(name=name, uops=lower(spec, ver=ver)).sha(ver)
    op = dops.DveOp(name, spec, subdim=False, uops_sha=shas)
    row = dops._CUSTOM_DVE_ROW_BASE + len(dops.OPS)
    dops.OPS.append(op)
    dops._SUB_OPCODE_FOR_NAME[name] = row
    dops.CUSTOM_DVE_SPECS[name] = spec

    # ---- hand-written 2X_1PORT program ----------------------------------
    # bf16 packed mode: each 32-bit port read carries two elements; the
    # crossbar exposes them as SRC_* (lo) and SRC_*_HI (hi). Stages 0-2
    # compute relu(in0_lo+s0)*in1_lo, stages 3-5 the hi half, stage 6 adds
    # the pair, stage 7 accumulates into CURR_ALU_OUT. Both products are
    # emitted packed via WR0_LO/WR0_HI. Exactly fills the 8-stage budget.
    from concourse.dve_uop import (
        ENABLE,
        AluInp,
        AluOp,
        DelayInp,
        InpSel,
        OutPath,
        OutSel,
        Trigger,
        UopConfig,
        UopDpConfig,
    )

    def _dp():
        return UopDpConfig()

    # seed uop: route accum_init (CONST_1) down delay lane 0, latch it into
    # stage 7's CURR_ALU_OUT; consumes no stream data, writes nothing.
    seed = UopConfig()
    seed.enable_input(InpSel.CONST_1, 1)
    for s in range(7):
        seed.datapath_config[s] = _dp().pass_through_alu().pass_through_delay(0)
    seed.datapath_config[7] = _dp().enable_alu(AluOp.BYPASS, AluInp.PREV_DELAY_0)
    seed.datapath_config[7].alu_out_a_enable = ENABLE
    seed.repeat_count = 1
    seed.trigger = (Trigger.COUNT, Trigger.NONE, Trigger.NONE)
    seed.next_uop = (1, 0, 0)

    st = UopConfig()
    st.enable_input(InpSel.SRC_0, 1)      # d0 = in0_lo
    st.enable_input(InpSel.SRC_0_HI, 2)   # d1 = in0_hi
    st.enable_input(InpSel.CONST_0, 3)    # d2 = s0
    st.enable_input(InpSel.ZERO, 4)       # d3 = 0
    st.enable_input(InpSel.SRC_1, 5)      # d4 = in1_lo
    st.enable_input(InpSel.SRC_1_HI, 6)   # d5 = in1_hi
    st.datapath_config[0] = (
        _dp()
        .enable_alu(AluOp.ADD, AluInp.PREV_DELAY_0, AluInp.PREV_DELAY_2)
        .pass_through_delay(1, 2, 3, 4, 5)
    )
    st.datapath_config[1] = (
        _dp()
        .enable_alu(AluOp.MAX, AluInp.PREV_ALU_OUT, AluInp.PREV_DELAY_3)
        .pass_through_delay(1, 2, 3, 4, 5)
    )
    st.datapath_config[2] = (
        _dp()
        .enable_alu(AluOp.MULTIPLY, AluInp.PREV_ALU_OUT, AluInp.PREV_DELAY_4)
        .pass_through_delay(1, 2, 3, 5)
    )
    st.datapath_config[3] = (
        _dp()
        .enable_alu(AluOp.ADD, AluInp.PREV_DELAY_1, AluInp.PREV_DELAY_2)
        .enable_delay_from_src(DelayInp.PREV_ALU_OUT, 0)  # capture p_lo
        .pass_through_delay(3, 5)
    )
    st.datapath_config[4] = (
        _dp()
        .enable_alu(AluOp.MAX, AluInp.PREV_ALU_OUT, AluInp.PREV_DELAY_3)
        .pass_through_delay(0, 5)
    )
    st.datapath_config[5] = (
        _dp()
        .enable_alu(AluOp.MULTIPLY, AluInp.PREV_ALU_OUT, AluInp.PREV_DELAY_5)
        .pass_through_delay(0)
    )
    st.datapath_config[6] = (
        _dp()
        .enable_alu(AluOp.ADD, AluInp.PREV_ALU_OUT, AluInp.PREV_DELAY_0)
        .enable_delay_from_src(DelayInp.PREV_ALU_OUT, 1)  # capture p_hi
        .pass_through_delay(0)
    )
    st.datapath_config[7] = (
        _dp()
        .enable_alu(AluOp.ADD, AluInp.CURR_ALU_OUT, AluInp.PREV_ALU_OUT)
        .pass_through_delay(0, 1)
    )
    st.datapath_config[7].alu_out_a_enable = ENABLE
    st.require_inp0 = ENABLE
    st.require_inp1 = ENABLE
    st.trigger = (Trigger.SRC_TENSOR_DONE, Trigger.NONE, Trigger.NONE)
    st.enable_output(OutSel.DELAY_0, OutPath.WR0_LO)
    st.enable_output(OutSel.DELAY_1, OutPath.WR0_HI)

    spec2x = DveOpSpec(
        name=name,
        opcode=row,
        uops=lower(spec, ver="v3"),
        uops_2x=[seed, st],
        perf_max=1,
        rd1_en=True,
    )
    spec2x.validate("v3")
    if os.environ.get("RMR_2X"):
        dops._COMPILE_CACHE[(name, "v3")] = spec2x
    return op


def _split_multi_waits(bir_bytes: bytes) -> bytes:
    """Walrus in this container allows at most ONE attached sync wait per
    instruction ("Too many sync wait commands").  Tile attaches several.
    Hoist extra waits onto standalone EventSemaphore instructions (which
    accept up to 2 waits each) inserted just before the instruction on the
    same engine stream.  Waits here are monotonic sem-ge waits, so
    satisfying them sequentially is equivalent to waiting on all at once.
    """
    import json

    bir = json.loads(bir_bytes)
    uid = 0
    for fn in bir["functions"]:
        for blk in fn["blocks"]:
            out = []
            for inst in blk["instructions"]:
                si = inst.get("sync_info")
                waits = (si or {}).get("on_wait") or []
                if len(waits) > 1:
                    extra, keep = waits[:-1], waits[-1:]
                    for k in range(0, len(extra), 1):
                        out.append(
                            {
                                "name": f"{inst['name']}-esw{uid}",
                                "opcode": "EventSemaphore",
                                "engine": inst["engine"],
                                "debug": inst.get("debug", 0),
                                "ins": [],
                                "outs": [],
                                "sync_info": {
                                    "on_update": [],
                                    "on_wait": extra[k : k + 1],
                                },
                            }
                        )
                        uid += 1
                    si["on_wait"] = keep
                out.append(inst)
            blk["instructions"] = out
    return json.dumps(bir).encode()


def build_nc() -> bass.Bass:
    rmr_op = _register_relu_mul_reduce()
    nc = bass.Bass()

    ent = nc.dram_tensor("entity", [B, N, D], FP32, kind="ExternalInput")
    ent_rows = nc.dram_tensor("entity_rows", [B, IS, D], FP32, kind="ExternalInput")
    edge_rows = nc.dram_tensor("edge_rows", [IS, N], FP32, kind="ExternalInput")
    u_rows = nc.dram_tensor("u_rows", [IS, N], FP32, kind="ExternalInput")
    Wp_d = nc.dram_tensor("Wp", [D, D], FP32, kind="ExternalInput")
    bp_d = nc.dram_tensor("bp", [D], FP32, kind="ExternalInput")
    W1s_d = nc.dram_tensor("W1s", [D, D], FP32, kind="ExternalInput")
    W1t_d = nc.dram_tensor("W1t", [D, D], FP32, kind="ExternalInput")
    b1_d = nc.dram_tensor("b1", [D], FP32, kind="ExternalInput")
    W2_d = nc.dram_tensor("W2", [D, D], FP32, kind="ExternalInput")
    b2_d = nc.dram_tensor("b2", [D], FP32, kind="ExternalInput")

    next_rows = nc.dram_tensor("next_rows", [B, IS, D], FP32, kind="ExternalOutput")
    a_rows = nc.dram_tensor("a_rows", [IS, N], FP32, kind="ExternalOutput")

    with tile.TileContext(nc) as tc, ExitStack() as ctx:
        consts = ctx.enter_context(tc.tile_pool(name="consts", bufs=1))
        perb = ctx.enter_context(tc.tile_pool(name="perb", bufs=1))
        work = ctx.enter_context(tc.tile_pool(name="work", bufs=3))
        astage = ctx.enter_context(tc.tile_pool(name="astage", bufs=1))
        arepl_pool = ctx.enter_context(tc.tile_pool(name="arepl", bufs=1))
        h_pool = ctx.enter_context(tc.tile_pool(name="h", bufs=8))
        psum = ctx.enter_context(tc.tile_pool(name="psum", bufs=3, space="PSUM"))
        psum_big = ctx.enter_context(
            tc.tile_pool(name="psum_big", bufs=2, space="PSUM")
        )
        dram = ctx.enter_context(tc.tile_pool(name="dram", bufs=1, space="DRAM"))

        # ---- adjacency inputs first: they head the critical path ----
        edge_t = astage.tile([IS, N], FP32, tag="edge")
        nc.sync.dma_start(edge_t[:], edge_rows[:, :])
        u_t = astage.tile([IS, N], FP32, tag="u")
        nc.sync.dma_start(u_t[:], u_rows[:, :])

        # ---- constants ----
        ident = consts.tile([P, P], FP32, tag="ident")
        make_identity(nc, ident)

        Wp_s = consts.tile([D, D], FP32, tag="Wp")
        nc.gpsimd.dma_start(Wp_s[:], Wp_d[:, :])
        W1s_s = consts.tile([D, D], FP32, tag="W1s")
        nc.gpsimd.dma_start(W1s_s[:], W1s_d[:, :])
        W1t_s = consts.tile([D, D], FP32, tag="W1t")
        nc.gpsimd.dma_start(W1t_s[:], W1t_d[:, :])

        w2f = work.tile([D, D], FP32, tag="w2f")
        nc.gpsimd.dma_start(w2f[:], W2_d[:, :])
        W2_bf = consts.tile([D, D], BF16, tag="W2bf")
        nc.vector.tensor_copy(W2_bf[:], w2f[:])

        bp_col = consts.tile([D, 1], FP32, tag="bp")
        nc.gpsimd.dma_start(bp_col[:], bp_d[:])
        b1_col = consts.tile([D, 1], FP32, tag="b1")
        nc.gpsimd.dma_start(b1_col[:], b1_d[:])

        b2f = work.tile([1, D], FP32, tag="b2f")
        b2_row_ap = bass.AP(tensor=b2_d[:].tensor, offset=0, ap=[[0, 1], [1, D]])
        nc.gpsimd.dma_start(b2f[:], b2_row_ap)
        b2_bf = consts.tile([1, D], BF16, tag="b2bf")
        nc.vector.tensor_copy(b2_bf[:], b2f[:])
        ones_bf = consts.tile([1, IS], BF16, tag="ones")
        nc.vector.memset(ones_bf[:], 1.0)

        # ---- adjacency: A = softmax((edge + gumbel(u)) / tau), fp32 ----
        eps_col = consts.tile([IS, 1], FP32, tag="epscol")
        nc.vector.memset(eps_col[:], EPS)
        l1 = astage.tile([IS, N], FP32, tag="l1")
        nc.scalar.activation(l1[:], u_t[:], AF.Ln, bias=eps_col[:], scale=1.0)
        l2 = astage.tile([IS, N], FP32, tag="l2")
        # log(eps - l1) = log(-log(u+eps) + eps)
        nc.scalar.activation(l2[:], l1[:], AF.Ln, bias=eps_col[:], scale=-1.0)
        # z = (edge - l2) * (1/tau); edge*(1/tau) runs parallel to the Lns
        edge2 = astage.tile([IS, N], FP32, tag="edge2")
        nc.vector.tensor_scalar_mul(edge2[:], edge_t[:], 1.0 / TAU)
        z2 = astage.tile([IS, N], FP32, tag="z2")
        nc.vector.scalar_tensor_tensor(
            z2[:], l2[:], -1.0 / TAU, edge2[:], op0=ALU.mult, op1=ALU.add
        )

        mneg = astage.tile([IS, 1], FP32, tag="mneg")
        nc.vector.tensor_reduce(
            mneg[:], z2[:], axis=mybir.AxisListType.X, op=ALU.max, negate=True
        )
        ex = astage.tile([IS, N], FP32, tag="ex")
        zsum = astage.tile([IS, 1], FP32, tag="zsum")
        nc.scalar.activation(
            ex[:], z2[:], AF.Exp, bias=mneg[:], scale=1.0, accum_out=zsum[:]
        )
        zinv = astage.tile([IS, 1], FP32, tag="zinv")
        nc.vector.reciprocal(zinv[:], zsum[:])
        a_f32 = astage.tile([IS, N], FP32, tag="af32")
        nc.vector.tensor_scalar_mul(a_f32[:], ex[:], zinv[:])
        nc.sync.dma_start(a_rows[:, :], a_f32[:])

        a_bf = astage.tile([IS, N], BF16, tag="abf")
        nc.vector.tensor_copy(a_bf[:], a_f32[:])
        a_scr = dram.tile([IS, N], BF16, tag="ascr")
        nc.gpsimd.dma_start(a_scr[:], a_bf[:])

        # ---- preload all A_repl broadcast tiles (row i of A replicated
        # across the 128 d-partitions), resident for the whole kernel ----
        arps = []
        for i in range(IS):
            arp = arepl_pool.tile([P, N], BF16, tag=f"arp{i}")
            nc.sync.dma_start(arp[:], a_scr[i : i + 1, :].to_broadcast((P, N)))
            arps.append(arp)

        # ---- per-batch: projection chain (PE), main loop, tail ----
        # b is the OUTER loop so batch b's units overlap batch b+1's
        # projection chain and batch b-1's tail.
        for b in range(B):
            # entity[b] transposed -> eT [d_in, 512]
            eT = work.tile([P, N], FP32, tag="eT")
            for k in range(4):
                ek = work.tile([P, D], FP32, tag="ek")
                nc.sync.dma_start(ek[:], ent[b, k * P : (k + 1) * P, :])
                tp = psum.tile([P, P], FP32, tag="ps_small")
                nc.tensor.transpose(tp[:], ek[:], ident[:])
                nc.vector.tensor_copy(eT[:, k * P : (k + 1) * P], tp[:])

            er_b = perb.tile([IS, D], FP32, tag=f"er{b}")
            nc.sync.dma_start(er_b[:], ent_rows[b, :, :])
            tpr = psum.tile([P, IS], FP32, tag="ps_small")
            nc.tensor.transpose(tpr[:], er_b[:], ident[:IS, :IS])
            eTr = work.tile([P, IS], FP32, tag="eTr")
            nc.vector.tensor_copy(eTr[:], tpr[:])

            pj = psum_big.tile([P, N], FP32, tag="ps_big")
            nc.tensor.matmul(pj[:], Wp_s[:], eT[:], start=True, stop=True)
            projT = work.tile([P, N], FP32, tag="projT")
            nc.scalar.activation(projT[:], pj[:], AF.Identity, bias=bp_col[:])

            pjr = psum.tile([P, IS], FP32, tag="ps_small")
            nc.tensor.matmul(pjr[:], Wp_s[:], eTr[:], start=True, stop=True)
            projTr = work.tile([P, IS], FP32, tag="projTr")
            nc.scalar.activation(projTr[:], pjr[:], AF.Identity, bias=bp_col[:])

            spx = psum_big.tile([P, N], FP32, tag="ps_big")
            nc.tensor.matmul(spx[:], W1s_s[:], projT[:], start=True, stop=True)
            sb1T_b = perb.tile([P, N], BF16, tag=f"sb1_{b}")
            nc.scalar.activation(sb1T_b[:], spx[:], AF.Identity, bias=b1_col[:])

            tpx = psum.tile([P, IS], FP32, tag="ps_small")
            nc.tensor.matmul(tpx[:], W1t_s[:], projTr[:], start=True, stop=True)
            tT_b = perb.tile([P, IS], FP32, tag=f"tT{b}")
            nc.vector.tensor_copy(tT_b[:], tpx[:])

            hagg_b = perb.tile([P, IS], FP32, tag=f"hagg{b}")

            # main loop: relu + A-weighted aggregation over sources
            for i in range(IS):
                idx = b * IS + i
                arp = arps[i]
                if _use_act(idx):
                    # ScalarE-assisted path: ACT relu -> DVE 2x bf16 mult ->
                    # ACT copy with fused free-dim accumulation.
                    h = h_pool.tile([P, N], BF16, tag="h")
                    nc.scalar.activation(
                        h[:], sb1T_b[:], AF.Relu, bias=tT_b[:, i : i + 1]
                    )
                    p = h_pool.tile([P, N], BF16, tag="p")
                    nc.vector.tensor_mul(p[:], h[:], arp[:])
                    scr = h_pool.tile([P, N], BF16, tag="scr")
                    nc.scalar.activation(
                        scr[:],
                        p[:],
                        AF.Copy,
                        bias=0.0,
                        accum_out=hagg_b[:, i : i + 1],
                    )
                else:
                    # Fully fused: one VectorE instruction does bias-add,
                    # relu, A-weighting, and the reduction over sources.
                    hw = h_pool.tile([P, N], BF16, tag="hw")
                    ci = nc.vector._custom_dve(
                        rmr_op,
                        out=hw[:],
                        in0=sb1T_b[:],
                        in1=arp[:],
                        s0=tT_b[:, i : i + 1],
                        s1=0.0,
                        accum_out=hagg_b[:, i : i + 1],
                    )
                    # perf_max left at 0: run the verified REGULAR program
                    _ = ci

            # tail: upd = hagg @ W2 + b2 ; next = entity_rows + upd
            hagg_bf = work.tile([P, IS], BF16, tag="haggbf")
            nc.vector.tensor_copy(hagg_bf[:], hagg_b[:])
            updp = psum.tile([IS, D], FP32, tag="ps_small")
            nc.tensor.matmul(updp[:], hagg_bf[:], W2_bf[:], start=True, stop=False)
            nc.tensor.matmul(updp[:], ones_bf[:], b2_bf[:], start=False, stop=True)
            nextt = work.tile([IS, D], FP32, tag="nextt")
            nc.vector.tensor_add(nextt[:], er_b[:], updp[:])
            nc.sync.dma_start(next_rows[b, :, :], nextt[:])

    # Lower InstISA subclasses (the custom DVE op) to raw instruction bytes;
    # Bacc.compile() does this but plain Bass+Tile does not.
    mybir.codegen_inst_isa_subclasses(nc)

    import types

    orig = type(nc).to_json_bytes
    nc.to_json_bytes = types.MethodType(
        lambda self: _split_multi_waits(orig(self)), nc
    )
    return nc


_CACHE: dict = {}


def kernel(entity_states, Wp, bp, edge_logits, W1, b1, W2, b2, u_noise):
    entity_states = np.ascontiguousarray(entity_states, dtype=np.float32)
    Wp = np.ascontiguousarray(Wp, dtype=np.float32)
    bp = np.ascontiguousarray(bp, dtype=np.float32)
    edge_logits = np.ascontiguousarray(edge_logits, dtype=np.float32)
    W1 = np.ascontiguousarray(W1, dtype=np.float32)
    b1 = np.ascontiguousarray(b1, dtype=np.float32)
    W2 = np.ascontiguousarray(W2, dtype=np.float32)
    b2 = np.ascontiguousarray(b2, dtype=np.float32)
    u_noise = np.ascontiguousarray(u_noise, dtype=np.float32)

    if "nc" not in _CACHE:
        _CACHE["nc"] = build_nc()
    nc = _CACHE["nc"]

    in_maps = []
    for c in range(M):
        sl = slice(c * IS, (c + 1) * IS)
        in_maps.append(
            {
                "entity": entity_states,
                "entity_rows": np.ascontiguousarray(entity_states[:, sl]),
                "edge_rows": np.ascontiguousarray(edge_logits[sl]),
                "u_rows": np.ascontiguousarray(u_noise[sl]),
                "Wp": Wp,
                "bp": bp,
                "W1s": np.ascontiguousarray(W1[:D]),
                "W1t": np.ascontiguousarray(W1[D:]),
                "b1": b1,
                "W2": W2,
                "b2": b2,
            }
        )

    res = run_bass_kernel_spmd(nc, in_maps, list(range(M)))
    results = res.results

    next_state = np.concatenate(
        [results[c]["next_rows"] for c in range(M)], axis=1
    ).astype(np.float32)
    a_full = np.concatenate([results[c]["a_rows"] for c in range(M)], axis=0).astype(
        np.float32
    )
    a_bcast = np.broadcast_to(a_full, (B, N, N)).copy()
    return next_state, a_bcast


# revision 24
# speedup vs baseline: 1.3649x; 1.0106x over previous
"""Trainium2 Bass kernel for nn_CausalSimulationModule (gnn message passing).

next_state = entity + (sum_j A[i,j] * relu(s[j] + t[i] + b1)) @ W2 + b2
A = softmax((edge_logits + gumbel(u)) / tau)   (row-wise over sources j)

Key algebraic move: W2 is linear, so aggregate h over sources j FIRST
(h-agg is (B,N,D)), then apply W2 once -- this removes the (B,N,N,D)@W2
matmul entirely.  The irreducible work is the 134M-element
relu + A-weighted reduction, done as:
  per (i, b):  H[d, j] = relu(sb1T[d, j] + tT[d, i])   (one fused op,
               split between ScalarE (activation bias) and VectorE
               (tensor_scalar add+max, 4x bf16 mode) to balance engines)
               hagg[:, i] = sum_j H * A_repl_i          (one fused DVE
               tensor_tensor_reduce with accum_out)
A_repl_i (row i of A broadcast across the 128 d-partitions) is produced
for free by a stride-0 partition-broadcast DMA from an HBM bf16 scratch.

Sharding: pure target-node sharding. Core c owns i in [c*64, (c+1)*64)
and loops all 4 batches internally. No collectives needed.
"""

import os
import sys

sys.path.insert(0, "/opt/trn_rl_repo")

from contextlib import ExitStack

import numpy as np

import concourse.bass as bass
import concourse.tile as tile
from concourse import mybir
from concourse.masks import make_identity
from concourse.bass_utils import run_bass_kernel_spmd

B, N, D = 4, 512, 128
M = 8            # NeuronCores
IS = N // M      # 64 target rows per core
TAU = 0.5
EPS = 1e-9
P = 128

FP32 = mybir.dt.float32
BF16 = mybir.dt.bfloat16
AF = mybir.ActivationFunctionType
ALU = mybir.AluOpType

# Fraction of the 256 (i, b) units routed through the ScalarE-assisted path
# (ACT relu -> DVE 2x bf16 multiply -> ACT copy+accum); the rest run as one
# fused custom DVE op. Balances ScalarE (~1427ns/unit) vs VectorE saving.
ACT_SHARE_NUM = 92
ACT_SHARE_DEN = 256


def _use_act(idx: int) -> bool:
    # Evenly interleaved split of units between the two paths.
    return (idx * ACT_SHARE_NUM) % ACT_SHARE_DEN < ACT_SHARE_NUM


def _register_relu_mul_reduce():
    """Author a custom fused DVE op:
        out = relu(in0 + s0) * in1 ; accum_out = s1 + sum_free(out)
    This collapses the whole per-(i,b) inner loop (bias-add, relu, A-weight
    multiply, reduction over sources) into ONE VectorE instruction.
    Registered by appending to concourse.dve_ops.OPS at runtime; the uop
    table is generated per-NEFF, so no firmware change is needed."""
    from operator import add as _add

    from concourse import dve_ops as dops
    from concourse.dve_spec import C0, C1, Spec, Src0, Src1, lower, relu
    from concourse.dve_uop import DveOpSpec

    name = "RELU_ADD_MUL_REDUCE_ANT"
    for o in dops.OPS:
        if o.name == name:
            return o

    def _ref(in0, in1, s0, s1, imm2):
        b = (np.maximum(in0.astype(np.float32) + s0, 0) * in1).astype(np.float32)
        return b, s1 + b.reshape(b.shape[0], -1).sum(axis=-1, keepdims=True)

    spec = Spec(body=relu(Src0 + C0) * Src1, accum=_add, accum_init=C1, reference=_ref)
    shas = {}
    for ver in ("v3", "v4"):
        shas[ver] = DveOpSpec(name=name, uops=lower(spec, ver=ver)).sha(ver)
    op = dops.DveOp(name, spec, subdim=False, uops_sha=shas)
    row = dops._CUSTOM_DVE_ROW_BASE + len(dops.OPS)
    dops.OPS.append(op)
    dops._SUB_OPCODE_FOR_NAME[name] = row
    dops.CUSTOM_DVE_SPECS[name] = spec

    # ---- hand-written 2X_1PORT program ----------------------------------
    # bf16 packed mode: each 32-bit port read carries two elements; the
    # crossbar exposes them as SRC_* (lo) and SRC_*_HI (hi). Stages 0-2
    # compute relu(in0_lo+s0)*in1_lo, stages 3-5 the hi half, stage 6 adds
    # the pair, stage 7 accumulates into CURR_ALU_OUT. Both products are
    # emitted packed via WR0_LO/WR0_HI. Exactly fills the 8-stage budget.
    from concourse.dve_uop import (
        ENABLE,
        AluInp,
        AluOp,
        DelayInp,
        InpSel,
        OutPath,
        OutSel,
        Trigger,
        UopConfig,
        UopDpConfig,
    )

    def _dp():
        return UopDpConfig()

    # seed uop: route accum_init (CONST_1) down delay lane 0, latch it into
    # stage 7's CURR_ALU_OUT; consumes no stream data, writes nothing.
    seed = UopConfig()
    seed.enable_input(InpSel.CONST_1, 1)
    for s in range(7):
        seed.datapath_config[s] = _dp().pass_through_alu().pass_through_delay(0)
    seed.datapath_config[7] = _dp().enable_alu(AluOp.BYPASS, AluInp.PREV_DELAY_0)
    seed.datapath_config[7].alu_out_a_enable = ENABLE
    seed.repeat_count = 1
    seed.trigger = (Trigger.COUNT, Trigger.NONE, Trigger.NONE)
    seed.next_uop = (1, 0, 0)

    st = UopConfig()
    st.enable_input(InpSel.SRC_0, 1)      # d0 = in0_lo
    st.enable_input(InpSel.SRC_0_HI, 2)   # d1 = in0_hi
    st.enable_input(InpSel.CONST_0, 3)    # d2 = s0
    st.enable_input(InpSel.ZERO, 4)       # d3 = 0
    st.enable_input(InpSel.SRC_1, 5)      # d4 = in1_lo
    st.enable_input(InpSel.SRC_1_HI, 6)   # d5 = in1_hi
    st.datapath_config[0] = (
        _dp()
        .enable_alu(AluOp.ADD, AluInp.PREV_DELAY_0, AluInp.PREV_DELAY_2)
        .pass_through_delay(1, 2, 3, 4, 5)
    )
    st.datapath_config[1] = (
        _dp()
        .enable_alu(AluOp.MAX, AluInp.PREV_ALU_OUT, AluInp.PREV_DELAY_3)
        .pass_through_delay(1, 2, 3, 4, 5)
    )
    st.datapath_config[2] = (
        _dp()
        .enable_alu(AluOp.MULTIPLY, AluInp.PREV_ALU_OUT, AluInp.PREV_DELAY_4)
        .pass_through_delay(1, 2, 3, 5)
    )
    st.datapath_config[3] = (
        _dp()
        .enable_alu(AluOp.ADD, AluInp.PREV_DELAY_1, AluInp.PREV_DELAY_2)
        .enable_delay_from_src(DelayInp.PREV_ALU_OUT, 0)  # capture p_lo
        .pass_through_delay(3, 5)
    )
    st.datapath_config[4] = (
        _dp()
        .enable_alu(AluOp.MAX, AluInp.PREV_ALU_OUT, AluInp.PREV_DELAY_3)
        .pass_through_delay(0, 5)
    )
    st.datapath_config[5] = (
        _dp()
        .enable_alu(AluOp.MULTIPLY, AluInp.PREV_ALU_OUT, AluInp.PREV_DELAY_5)
        .pass_through_delay(0)
    )
    st.datapath_config[6] = (
        _dp()
        .enable_alu(AluOp.ADD, AluInp.PREV_ALU_OUT, AluInp.PREV_DELAY_0)
        .enable_delay_from_src(DelayInp.PREV_ALU_OUT, 1)  # capture p_hi
        .pass_through_delay(0)
    )
    st.datapath_config[7] = (
        _dp()
        .enable_alu(AluOp.ADD, AluInp.CURR_ALU_OUT, AluInp.PREV_ALU_OUT)
        .pass_through_delay(0, 1)
    )
    st.datapath_config[7].alu_out_a_enable = ENABLE
    st.require_inp0 = ENABLE
    st.require_inp1 = ENABLE
    st.trigger = (Trigger.SRC_TENSOR_DONE, Trigger.NONE, Trigger.NONE)
    st.enable_output(OutSel.DELAY_0, OutPath.WR0_LO)
    st.enable_output(OutSel.DELAY_1, OutPath.WR0_HI)

    spec2x = DveOpSpec(
        name=name,
        opcode=row,
        uops=lower(spec, ver="v3"),
        uops_2x=[seed, st],
        perf_max=1,
        rd1_en=True,
    )
    spec2x.validate("v3")
    if os.environ.get("RMR_2X"):
        dops._COMPILE_CACHE[(name, "v3")] = spec2x
    return op


def _split_multi_waits(bir_bytes: bytes) -> bytes:
    """Walrus in this container allows at most ONE attached sync wait per
    instruction ("Too many sync wait commands").  Tile attaches several.
    Hoist extra waits onto standalone EventSemaphore instructions (which
    accept up to 2 waits each) inserted just before the instruction on the
    same engine stream.  Waits here are monotonic sem-ge waits, so
    satisfying them sequentially is equivalent to waiting on all at once.
    """
    import json

    bir = json.loads(bir_bytes)
    uid = 0
    for fn in bir["functions"]:
        for blk in fn["blocks"]:
            out = []
            for inst in blk["instructions"]:
                si = inst.get("sync_info")
                waits = (si or {}).get("on_wait") or []
                if len(waits) > 1:
                    extra, keep = waits[:-1], waits[-1:]
                    for k in range(0, len(extra), 1):
                        out.append(
                            {
                                "name": f"{inst['name']}-esw{uid}",
                                "opcode": "EventSemaphore",
                                "engine": inst["engine"],
                                "debug": inst.get("debug", 0),
                                "ins": [],
                                "outs": [],
                                "sync_info": {
                                    "on_update": [],
                                    "on_wait": extra[k : k + 1],
                                },
                            }
                        )
                        uid += 1
                    si["on_wait"] = keep
                out.append(inst)
            blk["instructions"] = out
    return json.dumps(bir).encode()


def build_nc() -> bass.Bass:
    rmr_op = _register_relu_mul_reduce()
    nc = bass.Bass()

    ent = nc.dram_tensor("entity", [B, N, D], FP32, kind="ExternalInput")
    ent_rows = nc.dram_tensor("entity_rows", [B, IS, D], FP32, kind="ExternalInput")
    edge_rows = nc.dram_tensor("edge_rows", [IS, N], FP32, kind="ExternalInput")
    u_rows = nc.dram_tensor("u_rows", [IS, N], FP32, kind="ExternalInput")
    Wp_d = nc.dram_tensor("Wp", [D, D], FP32, kind="ExternalInput")
    bp_d = nc.dram_tensor("bp", [D], FP32, kind="ExternalInput")
    W1s_d = nc.dram_tensor("W1s", [D, D], FP32, kind="ExternalInput")
    W1t_d = nc.dram_tensor("W1t", [D, D], FP32, kind="ExternalInput")
    b1_d = nc.dram_tensor("b1", [D], FP32, kind="ExternalInput")
    W2_d = nc.dram_tensor("W2", [D, D], FP32, kind="ExternalInput")
    b2_d = nc.dram_tensor("b2", [D], FP32, kind="ExternalInput")

    next_rows = nc.dram_tensor("next_rows", [B, IS, D], FP32, kind="ExternalOutput")
    a_rows = nc.dram_tensor("a_rows", [IS, N], FP32, kind="ExternalOutput")

    with tile.TileContext(nc) as tc, ExitStack() as ctx:
        consts = ctx.enter_context(tc.tile_pool(name="consts", bufs=1))
        perb = ctx.enter_context(tc.tile_pool(name="perb", bufs=1))
        work = ctx.enter_context(tc.tile_pool(name="work", bufs=3))
        astage = ctx.enter_context(tc.tile_pool(name="astage", bufs=1))
        arepl_pool = ctx.enter_context(tc.tile_pool(name="arepl", bufs=1))
        h_pool = ctx.enter_context(tc.tile_pool(name="h", bufs=8))
        psum = ctx.enter_context(tc.tile_pool(name="psum", bufs=3, space="PSUM"))
        psum_big = ctx.enter_context(
            tc.tile_pool(name="psum_big", bufs=2, space="PSUM")
        )
        dram = ctx.enter_context(tc.tile_pool(name="dram", bufs=1, space="DRAM"))

        # ---- adjacency inputs first: they head the critical path ----
        edge_t = astage.tile([IS, N], FP32, tag="edge")
        nc.sync.dma_start(edge_t[:], edge_rows[:, :])
        u_t = astage.tile([IS, N], FP32, tag="u")
        nc.sync.dma_start(u_t[:], u_rows[:, :])

        # ---- constants ----
        ident = consts.tile([P, P], FP32, tag="ident")
        make_identity(nc, ident)

        Wp_s = consts.tile([D, D], FP32, tag="Wp")
        nc.gpsimd.dma_start(Wp_s[:], Wp_d[:, :])
        W1s_s = consts.tile([D, D], FP32, tag="W1s")
        nc.gpsimd.dma_start(W1s_s[:], W1s_d[:, :])
        W1t_s = consts.tile([D, D], FP32, tag="W1t")
        nc.gpsimd.dma_start(W1t_s[:], W1t_d[:, :])

        w2f = work.tile([D, D], FP32, tag="w2f")
        nc.gpsimd.dma_start(w2f[:], W2_d[:, :])
        W2_bf = consts.tile([D, D], BF16, tag="W2bf")
        nc.vector.tensor_copy(W2_bf[:], w2f[:])

        bp_col = consts.tile([D, 1], FP32, tag="bp")
        nc.gpsimd.dma_start(bp_col[:], bp_d[:])
        b1_col = consts.tile([D, 1], FP32, tag="b1")
        nc.gpsimd.dma_start(b1_col[:], b1_d[:])

        b2f = work.tile([1, D], FP32, tag="b2f")
        b2_row_ap = bass.AP(tensor=b2_d[:].tensor, offset=0, ap=[[0, 1], [1, D]])
        nc.gpsimd.dma_start(b2f[:], b2_row_ap)
        b2_bf = consts.tile([1, D], BF16, tag="b2bf")
        nc.vector.tensor_copy(b2_bf[:], b2f[:])
        ones_bf = consts.tile([1, IS], BF16, tag="ones")
        nc.vector.memset(ones_bf[:], 1.0)

        # ---- adjacency: A = softmax((edge + gumbel(u)) / tau), fp32 ----
        eps_col = consts.tile([IS, 1], FP32, tag="epscol")
        nc.vector.memset(eps_col[:], EPS)
        l1 = astage.tile([IS, N], FP32, tag="l1")
        nc.scalar.activation(l1[:], u_t[:], AF.Ln, bias=eps_col[:], scale=1.0)
        l2 = astage.tile([IS, N], FP32, tag="l2")
        # log(eps - l1) = log(-log(u+eps) + eps)
        nc.scalar.activation(l2[:], l1[:], AF.Ln, bias=eps_col[:], scale=-1.0)
        # z = (edge - l2) * (1/tau); edge*(1/tau) runs parallel to the Lns
        edge2 = astage.tile([IS, N], FP32, tag="edge2")
        nc.vector.tensor_scalar_mul(edge2[:], edge_t[:], 1.0 / TAU)
        z2 = astage.tile([IS, N], FP32, tag="z2")
        nc.vector.scalar_tensor_tensor(
            z2[:], l2[:], -1.0 / TAU, edge2[:], op0=ALU.mult, op1=ALU.add
        )

        mneg = astage.tile([IS, 1], FP32, tag="mneg")
        nc.vector.tensor_reduce(
            mneg[:], z2[:], axis=mybir.AxisListType.X, op=ALU.max, negate=True
        )
        ex = astage.tile([IS, N], FP32, tag="ex")
        zsum = astage.tile([IS, 1], FP32, tag="zsum")
        nc.scalar.activation(
            ex[:], z2[:], AF.Exp, bias=mneg[:], scale=1.0, accum_out=zsum[:]
        )
        zinv = astage.tile([IS, 1], FP32, tag="zinv")
        nc.vector.reciprocal(zinv[:], zsum[:])
        a_f32 = astage.tile([IS, N], FP32, tag="af32")
        nc.vector.tensor_scalar_mul(a_f32[:], ex[:], zinv[:])
        nc.sync.dma_start(a_rows[:, :], a_f32[:])

        a_bf = astage.tile([IS, N], BF16, tag="abf")
        nc.vector.tensor_copy(a_bf[:], a_f32[:])
        a_scr = dram.tile([IS, N], BF16, tag="ascr")
        nc.gpsimd.dma_start(a_scr[:], a_bf[:])

        # ---- preload all A_repl broadcast tiles (row i of A replicated
        # across the 128 d-partitions), resident for the whole kernel ----
        arps = []
        for i in range(IS):
            arp = arepl_pool.tile([P, N], BF16, tag=f"arp{i}")
            nc.sync.dma_start(arp[:], a_scr[i : i + 1, :].to_broadcast((P, N)))
            arps.append(arp)

        # ---- per-batch: projection chain (PE), main loop, tail ----
        # b is the OUTER loop so batch b's units overlap batch b+1's
        # projection chain and batch b-1's tail.
        for b in range(B):
            # entity[b] transposed -> eT [d_in, 512]
            eT = work.tile([P, N], FP32, tag="eT")
            for k in range(4):
                ek = work.tile([P, D], FP32, tag="ek")
                nc.sync.dma_start(ek[:], ent[b, k * P : (k + 1) * P, :])
                tp = psum.tile([P, P], FP32, tag="ps_small")
                nc.tensor.transpose(tp[:], ek[:], ident[:])
                nc.vector.tensor_copy(eT[:, k * P : (k + 1) * P], tp[:])

            er_b = perb.tile([IS, D], FP32, tag=f"er{b}")
            nc.sync.dma_start(er_b[:], ent_rows[b, :, :])
            tpr = psum.tile([P, IS], FP32, tag="ps_small")
            nc.tensor.transpose(tpr[:], er_b[:], ident[:IS, :IS])
            eTr = work.tile([P, IS], FP32, tag="eTr")
            nc.vector.tensor_copy(eTr[:], tpr[:])

            pj = psum_big.tile([P, N], FP32, tag="ps_big")
            nc.tensor.matmul(pj[:], Wp_s[:], eT[:], start=True, stop=True)
            projT = work.tile([P, N], FP32, tag="projT")
            nc.scalar.activation(projT[:], pj[:], AF.Identity, bias=bp_col[:])

            pjr = psum.tile([P, IS], FP32, tag="ps_small")
            nc.tensor.matmul(pjr[:], Wp_s[:], eTr[:], start=True, stop=True)
            projTr = work.tile([P, IS], FP32, tag="projTr")
            nc.scalar.activation(projTr[:], pjr[:], AF.Identity, bias=bp_col[:])

            spx = psum_big.tile([P, N], FP32, tag="ps_big")
            nc.tensor.matmul(spx[:], W1s_s[:], projT[:], start=True, stop=True)
            sb1T_b = perb.tile([P, N], BF16, tag=f"sb1_{b}")
            nc.scalar.activation(sb1T_b[:], spx[:], AF.Identity, bias=b1_col[:])

            tpx = psum.tile([P, IS], FP32, tag="ps_small")
            nc.tensor.matmul(tpx[:], W1t_s[:], projTr[:], start=True, stop=True)
            tT_b = perb.tile([P, IS], FP32, tag=f"tT{b}")
            nc.vector.tensor_copy(tT_b[:], tpx[:])

            hagg_b = perb.tile([P, IS], FP32, tag=f"hagg{b}")

            # main loop: relu + A-weighted aggregation over sources
            for i in range(IS):
                idx = b * IS + i
                arp = arps[i]
                if _use_act(idx):
                    # ScalarE-assisted path: ACT relu -> DVE 2x bf16 mult ->
                    # ACT copy with fused free-dim accumulation.
                    h = h_pool.tile([P, N], BF16, tag="h")
                    nc.scalar.activation(
                        h[:], sb1T_b[:], AF.Relu, bias=tT_b[:, i : i + 1]
                    )
                    p = h_pool.tile([P, N], BF16, tag="p")
                    nc.vector.tensor_mul(p[:], h[:], arp[:])
                    scr = h_pool.tile([P, N], BF16, tag="scr")
                    nc.scalar.activation(
                        scr[:],
                        p[:],
                        AF.Copy,
                        bias=0.0,
                        accum_out=hagg_b[:, i : i + 1],
                    )
                else:
                    # Fully fused: one VectorE instruction does bias-add,
                    # relu, A-weighting, and the reduction over sources.
                    hw = h_pool.tile([P, N], BF16, tag="hw")
                    ci = nc.vector._custom_dve(
                        rmr_op,
                        out=hw[:],
                        in0=sb1T_b[:],
                        in1=arp[:],
                        s0=tT_b[:, i : i + 1],
                        s1=0.0,
                        accum_out=hagg_b[:, i : i + 1],
                    )
                    # perf_max left at 0: run the verified REGULAR program
                    _ = ci

            # tail: upd = hagg @ W2 + b2 ; next = entity_rows + upd
            hagg_bf = work.tile([P, IS], BF16, tag="haggbf")
            nc.vector.tensor_copy(hagg_bf[:], hagg_b[:])
            updp = psum.tile([IS, D], FP32, tag="ps_small")
            nc.tensor.matmul(updp[:], hagg_bf[:], W2_bf[:], start=True, stop=False)
            nc.tensor.matmul(updp[:], ones_bf[:], b2_bf[:], start=False, stop=True)
            nextt = work.tile([IS, D], FP32, tag="nextt")
            nc.vector.tensor_add(nextt[:], er_b[:], updp[:])
            nc.sync.dma_start(next_rows[b, :, :], nextt[:])

    # Lower InstISA subclasses (the custom DVE op) to raw instruction bytes;
    # Bacc.compile() does this but plain Bass+Tile does not.
    mybir.codegen_inst_isa_subclasses(nc)

    import types

    orig = type(nc).to_json_bytes
    nc.to_json_bytes = types.MethodType(
        lambda self: _split_multi_waits(orig(self)), nc
    )
    return nc


_CACHE: dict = {}


def kernel(entity_states, Wp, bp, edge_logits, W1, b1, W2, b2, u_noise):
    entity_states = np.ascontiguousarray(entity_states, dtype=np.float32)
    Wp = np.ascontiguousarray(Wp, dtype=np.float32)
    bp = np.ascontiguousarray(bp, dtype=np.float32)
    edge_logits = np.ascontiguousarray(edge_logits, dtype=np.float32)
    W1 = np.ascontiguousarray(W1, dtype=np.float32)
    b1 = np.ascontiguousarray(b1, dtype=np.float32)
    W2 = np.ascontiguousarray(W2, dtype=np.float32)
    b2 = np.ascontiguousarray(b2, dtype=np.float32)
    u_noise = np.ascontiguousarray(u_noise, dtype=np.float32)

    if "nc" not in _CACHE:
        _CACHE["nc"] = build_nc()
    nc = _CACHE["nc"]

    in_maps = []
    for c in range(M):
        sl = slice(c * IS, (c + 1) * IS)
        in_maps.append(
            {
                "entity": entity_states,
                "entity_rows": np.ascontiguousarray(entity_states[:, sl]),
                "edge_rows": np.ascontiguousarray(edge_logits[sl]),
                "u_rows": np.ascontiguousarray(u_noise[sl]),
                "Wp": Wp,
                "bp": bp,
                "W1s": np.ascontiguousarray(W1[:D]),
                "W1t": np.ascontiguousarray(W1[D:]),
                "b1": b1,
                "W2": W2,
                "b2": b2,
            }
        )

    res = run_bass_kernel_spmd(nc, in_maps, list(range(M)))
    results = res.results

    next_state = np.concatenate(
        [results[c]["next_rows"] for c in range(M)], axis=1
    ).astype(np.float32)
    a_full = np.concatenate([results[c]["a_rows"] for c in range(M)], axis=0).astype(
        np.float32
    )
    a_bcast = np.broadcast_to(a_full, (B, N, N)).copy()
    return next_state, a_bcast
